# revision 2
# baseline (speedup 1.0000x reference)
"""MixtureOfDepth transformer on 8 trn2 NeuronCores (Bass/Tile).

DP-4 over batch x TP-2 within core pairs. Speed strategy vs the fp32
baseline:
  * All weight GEMMs (QKV, Wo, W1, W2) and attention scores run as 3-term
    split-fp32r matmuls (W = Whi + Wlo, x = xhi + xlo at a 12-significand-bit
    boundary; W@x ~= Whi@xhi + Whi@xlo + Wlo@xhi). fp32r streams at 1
    cycle/row @2.4GHz vs fp32's 2 half-speed passes, so 3 passes beat fp32's
    effective 4 -- at fp32-identical accuracy (validated on HW: 1.7e-7 both).
  * Bias adds via scalar-engine activation bias columns (kills all K=1 bias
    broadcast matmuls).
  * LayerNorm per-token stats math runs in [Tl/128,128] column form (full DVE
    width) instead of [1,512] rows; scale/shift broadcasts are 2-term fp32r.
  * Layer 5 (everything after the last routing decision) runs plain fp32r.
AV (attention @ V) and v-proj stay fp32: their moving operands (softmax
probs) can't be split cheaply. Routing decisions stay exact (fp32 trunk
accuracy everywhere a later top-k depends on it; boundary gaps are as small
as 1.8e-5).
"""
import os, sys
import numpy as np

sys.path.insert(0, "/opt/trn_rl_repo")
import concourse.bass as bass
import concourse.tile as tile
from concourse import bacc, mybir
from concourse import bass_utils
from contextlib import ExitStack

FP = mybir.dt.float32
FPR = mybir.dt.float32r
I32 = mybir.dt.int32

D, H, HD, DFF, NL, T, B = 1024, 16, 64, 4096, 6, 2048, 4
EPS = 1e-5
HH, DFH, KSEL = H // 2, 4096 // 2, T // 2
AF = mybir.ActivationFunctionType
OP = mybir.AluOpType
RG = [[0, 1], [2, 3], [4, 5], [6, 7]]
IMASK = -1048576  # 0xFFFFF000 as int32: keep 12 significand bits

_CACHED = {}


class Ctr:
    def __init__(self):
        self.i = 0

    def nm(self, p):
        self.i += 1
        return f"{p}{self.i}"


def emit_split(nc, hi, lo, src):
    """hi = fp32r-rounding of src (DVE rounding copy); lo = src - hi.

    hi/lo must be fp32r APs, src fp32. src = hi + lo exactly for any
    hardware fp32r mantissa width, and both parts are exactly representable
    in fp32r, so split-term matmuls reconstruct fp32 accuracy."""
    nc.vector.tensor_copy(hi, src)
    nc.vector.tensor_tensor(lo, src.bitcast(FPR), hi, op=OP.subtract)


def emit_ln(nc, tc, u, li, Tl, x_dram, add_dram, g_col, b_col, C, dram,
            out_dram, mode="f"):
    """out <- LN(x + add) * g + b. x/add read from DRAM [D,Tl]; out to DRAM."""
    split = (mode == "f")
    NT = Tl // 512
    NP = Tl // 128
    es = ExitStack()
    sb = es.enter_context(tc.tile_pool(name=u.nm("lnsb"), bufs=2))
    tp = es.enter_context(tc.tile_pool(name=u.nm("lnt"), bufs=1))
    stat_d = dram.tile([1, 2 * Tl], FP, name=u.nm("statd"))
    rs_d = dram.tile([4, Tl], FP, name=u.nm("rsd"))
    t_tiles = []
    esPA = ExitStack()
    psA = esPA.enter_context(tc.tile_pool(name=u.nm("lnpsA"), bufs=1, space="PSUM"))
    a1 = [psA.tile([1, 512], FP, tag=f"r1_{tb}", bufs=1, name=u.nm("r1"))
          for tb in range(NT)]
    a2 = [psA.tile([1, 512], FP, tag=f"r2_{tb}", bufs=1, name=u.nm("r2"))
          for tb in range(NT)]
    ones_col = C["ones_col"]
    for dc in range(8):
        t = tp.tile([128, Tl], FP, tag=f"lt{dc}", bufs=1, name=u.nm("lt"))
        nc.sync.dma_start(t[:], x_dram[128 * dc:128 * (dc + 1), :])
        a = sb.tile([128, Tl], FP, tag="lnadd", bufs=2, name=u.nm("la"))
        nc.sync.dma_start(a[:], add_dram[128 * dc:128 * (dc + 1), :])
        nc.vector.tensor_tensor(t[:], t[:], a[:], op=OP.add)
        x2 = sb.tile([128, Tl], FP, tag="x2", bufs=2, name=u.nm("x2"))
        nc.scalar.square(x2[:], t[:])
        for tb in range(NT):
            sl = slice(512 * tb, 512 * (tb + 1))
            nc.tensor.matmul(a1[tb][:], ones_col[:, 0:1], t[:, sl],
                             start=(dc == 0), stop=(dc == 7))
            nc.tensor.matmul(a2[tb][:], ones_col[:, 0:1], x2[:, sl],
                             start=(dc == 0), stop=(dc == 7))
        t_tiles.append(t)
    # stats to a row, then to column form [2*NP, 128]
    sxq = sb.tile([1, 2 * Tl], FP, tag="sxq", bufs=1, name=u.nm("sxq"))
    for tb in range(NT):
        nc.vector.tensor_copy(sxq[0:1, 512 * tb:512 * (tb + 1)], a1[tb][:])
        nc.vector.tensor_copy(sxq[0:1, Tl + 512 * tb:Tl + 512 * (tb + 1)],
                              a2[tb][:])
    esPA.close()
    nc.sync.dma_start(stat_d[0:1, :], sxq[:])
    sxcol = sb.tile([NP, 128], FP, tag="sxcol", bufs=1, name=u.nm("sxcol"))
    nc.sync.dma_start(sxcol[:],
                      stat_d[0:1, 0:Tl].rearrange("a (b c) -> (a b) c", c=128))
    sqcol = sb.tile([NP, 128], FP, tag="sqcol", bufs=1, name=u.nm("sqcol"))
    nc.sync.dma_start(sqcol[:],
                      stat_d[0:1, Tl:2 * Tl].rearrange("a (b c) -> (a b) c",
                                                       c=128))
    sx = sxcol[:]
    sq = sqcol[:]

    def ctile(nm):
        return sb.tile([NP, 128], FP, tag=nm, bufs=1, name=u.nm(nm))

    mu = ctile("mu")
    nc.vector.tensor_scalar(mu[:], sx, 1.0 / D, None, OP.mult)
    veps = ctile("veps")
    nc.vector.tensor_scalar(veps[:], sq, 1.0 / D, None, OP.mult)
    mu2 = ctile("mu2")
    nc.vector.tensor_tensor(mu2[:], mu[:], mu[:], op=OP.mult)
    nc.vector.tensor_tensor(veps[:], veps[:], mu2[:], op=OP.subtract)
    nc.vector.tensor_scalar(veps[:], veps[:], EPS, None, OP.add)
    s0 = ctile("s0")
    nc.scalar.sqrt(s0[:], veps[:])
    r0 = ctile("r0")
    nc.vector.reciprocal(r0[:], s0[:])
    t1 = ctile("t1")
    nc.vector.tensor_tensor(t1[:], r0[:], r0[:], op=OP.mult)
    nc.vector.tensor_tensor(t1[:], t1[:], veps[:], op=OP.mult)
    nc.vector.tensor_scalar(t1[:], t1[:], -0.5, 1.5, OP.mult, OP.add)
    rs = ctile("rs")
    nc.vector.tensor_tensor(rs[:], r0[:], t1[:], op=OP.mult)
    nmrs = ctile("nmrs")
    nc.vector.tensor_tensor(nmrs[:], mu[:], rs[:], op=OP.mult)
    nc.vector.tensor_scalar(nmrs[:], nmrs[:], -1.0, None, OP.mult)
    # split scale/shift (column form), stage to DRAM, reload as fp32r rows
    def rtile(nm):
        return sb.tile([NP, 128], FPR, tag=nm, bufs=1, name=u.nm(nm))

    if split:
        rsh, rsl, nmh, nml = rtile("rsh"), rtile("rsl"), rtile("nmh"), rtile("nml")
        emit_split(nc, rsh[:], rsl[:], rs[:])
        emit_split(nc, nmh[:], nml[:], nmrs[:])
        quads = [rsh, rsl, nmh, nml]
    else:
        quads = [rtile("rsh"), rtile("nmh")]
        nc.vector.tensor_copy(quads[0][:], rs[:])
        nc.vector.tensor_copy(quads[1][:], nmrs[:])
    rows = []
    for qi, q in enumerate(quads):
        nc.sync.dma_start(
            rs_d[qi:qi + 1, :].rearrange("a (b c) -> (a b) c",
                                         c=128).bitcast(FPR), q[:])
        r = sb.tile([1, Tl], FPR, tag=f"lnrow{qi}", bufs=1, name=u.nm("rows"))
        nc.sync.dma_start(r[:], rs_d[qi:qi + 1, :].bitcast(FPR))
        rows.append(r)
    ones_r = C["ones_row_r"]
    esPB = ExitStack()
    psB = esPB.enter_context(tc.tile_pool(name=u.nm("lnpsB"), bufs=1, space="PSUM"))
    for tb in range(NT):
        sl = slice(512 * tb, 512 * (tb + 1))
        b1p = psB.tile([128, 512], FP, tag="bc1", bufs=2, name=u.nm("b1p"))
        b2p = psB.tile([128, 512], FP, tag="bc2", bufs=2, name=u.nm("b2p"))
        if split:
            nc.tensor.matmul(b1p[:], ones_r[0:1, 0:128], rows[0][0:1, sl],
                             start=True, stop=False)
            nc.tensor.matmul(b1p[:], ones_r[0:1, 0:128], rows[1][0:1, sl],
                             start=False, stop=True)
            nc.tensor.matmul(b2p[:], ones_r[0:1, 0:128], rows[2][0:1, sl],
                             start=True, stop=False)
            nc.tensor.matmul(b2p[:], ones_r[0:1, 0:128], rows[3][0:1, sl],
                             start=False, stop=True)
        else:
            nc.tensor.matmul(b1p[:], ones_r[0:1, 0:128], rows[0][0:1, sl],
                             start=True, stop=True)
            nc.tensor.matmul(b2p[:], ones_r[0:1, 0:128], rows[1][0:1, sl],
                             start=True, stop=True)
        for dc in range(8):
            v1 = sb.tile([128, 512], FP, tag="v1", bufs=3, name=u.nm("v1"))
            nc.vector.tensor_tensor(v1[:], t_tiles[dc][:, sl], b1p[:],
                                    op=OP.mult)
            nc.vector.tensor_tensor(v1[:], v1[:], b2p[:], op=OP.add)
            o1 = sb.tile([128, 512], FP, tag="o1", bufs=3, name=u.nm("o1"))
            nc.scalar.activation(o1[:], v1[:], AF.Identity,
                                 bias=b_col[:, dc:dc + 1],
                                 scale=g_col[:, dc:dc + 1])
            nc.sync.dma_start(out_dram[128 * dc:128 * (dc + 1), sl], o1[:])
    esPB.close()
    es.close()


def emit_encoder(nc, tc, u, li, Tl, x_dram, W, C, dram, out_dram, mode="f"):
    """Encoder layer reading x from DRAM [D, Tl], writing new x to out_dram."""
    split = (mode == "f")
    NT = Tl // 512
    NTC = Tl // 128
    ar1 = dram.tile([D, Tl], FP, name=u.nm("ar1i"))
    ar1o = dram.tile([D, Tl], FP, name=u.nm("ar1o"))
    xhi_d = dram.tile([D, Tl], FP, name=u.nm("xhid"))
    xlo_d = dram.tile([D, Tl], FP, name=u.nm("xlod"))
    ones_row = C["ones_row"]

    # ---- phase 0: v-projection for all groups + x hi/lo staging ----
    esA = ExitStack()
    oTp = esA.enter_context(tc.tile_pool(name=u.nm("aoT"), bufs=1))
    oTn = [oTp.tile([128, Tl], FP if split else FPR, tag=f"oT{i}", bufs=1,
                    name=u.nm("oT"))
           for i in range(4)]
    esasb = ExitStack()
    asb = esasb.enter_context(tc.tile_pool(name=u.nm("asb"), bufs=2))
    es0 = ExitStack()
    vap = es0.enter_context(tc.tile_pool(name=u.nm("avA"), bufs=1))
    VDT = FP if split else FPR
    vA = [[vap.tile([128, 130], VDT, tag=f"vA{g}_{i % 4}",
                    bufs=(NTC + 3) // 4, name=u.nm("vA"))
           for i in range(NTC)] for g in range(4)]
    esV = ExitStack()
    vsb = esV.enter_context(tc.tile_pool(name=u.nm("vsb"), bufs=2))
    vps = esV.enter_context(tc.tile_pool(name=u.nm("vps"), bufs=1, space="PSUM"))
    wv = []
    for dc in range(8):
        t = vsb.tile([128, 512], VDT, tag=f"wv{dc}", bufs=1, name=u.nm("wv"))
        src = W["wv_rows"][li, dc]
        nc.sync.dma_start(t[:], src if split else src.bitcast(FPR))
        wv.append(t)
    ones_c = C["ones_col"]
    for tb in range(NT):
        sl = slice(512 * tb, 512 * (tb + 1))
        xc = []
        for dc in range(8):
            t = vsb.tile([128, 512], FP, tag=f"xc{dc}", bufs=1,
                         name=u.nm("xc"))
            nc.sync.dma_start(t[:], x_dram[128 * dc:128 * (dc + 1), sl])
            xc.append(t)
        if split:
            for dc in range(8):
                hi = vsb.tile([128, 512], FPR, tag="xh", bufs=4,
                              name=u.nm("xh"))
                lo = vsb.tile([128, 512], FPR, tag="xl", bufs=4,
                              name=u.nm("xl"))
                emit_split(nc, hi[:], lo[:], xc[dc][:])
                nc.sync.dma_start(xhi_d[128 * dc:128 * (dc + 1), sl],
                                  hi[:].bitcast(FP))
                nc.sync.dma_start(xlo_d[128 * dc:128 * (dc + 1), sl],
                                  lo[:].bitcast(FP))
        for g in range(4):
            vs = slice(128 * g, 128 * (g + 1))
            for ti in range(4):
                tig = 4 * tb + ti
                acc = vps.tile([128, 128], FP, tag="vacc", bufs=2,
                               name=u.nm("va"))
                for dc in range(8):
                    xop = xc[dc][:, 128 * ti:128 * (ti + 1)]
                    nc.tensor.matmul(acc[:], xop if split else xop.bitcast(FPR),
                                     wv[dc][:, vs], start=(dc == 0),
                                     stop=(dc == 7))
                nc.vector.tensor_copy(vA[g][tig][:, 64:65], ones_c[:, 0:1])
                nc.vector.tensor_copy(vA[g][tig][:, 129:130], ones_c[:, 0:1])
                src = acc[:, :].rearrange("p (h c) -> p h c", c=64)
                dst = vA[g][tig][:, :].rearrange("p (h c) -> p h c",
                                                 c=65)[:, :, 0:64]
                nc.vector.tensor_copy(dst, src)
    esV.close()

    # ---- attention per 2-head group ----
    for g in range(4):
        esG = ExitStack()
        gp = esG.enter_context(tc.tile_pool(name=u.nm("gq"), bufs=1))
        ps = esG.enter_context(tc.tile_pool(name=u.nm("gps"), bufs=1, space="PSUM"))
        if split:
            qk = tuple(
                (gp.tile([128, Tl], FPR, tag=f"q{r}h", bufs=1, name=u.nm("qh")),
                 gp.tile([128, Tl], FPR, tag=f"q{r}l", bufs=1, name=u.nm("ql")))
                for r in range(2))
        else:
            qk = tuple(
                (gp.tile([128, Tl], FPR, tag=f"q{r}h", bufs=1, name=u.nm("qT")),)
                for r in range(2))
        # weights for q (cc=g) and k (cc=4+g)
        wq = []
        for role in range(2):
            cc = g if role == 0 else 4 + g
            wth = asb.tile([128, 1024], FPR, tag=f"wqh{role}", bufs=1,
                           name=u.nm("wqh"))
            nc.sync.dma_start(wth[:], W["wqkv_hi"][li, cc].bitcast(FPR))
            if split:
                wtl = asb.tile([128, 1024], FPR, tag=f"wql{role}", bufs=1,
                               name=u.nm("wql"))
                nc.sync.dma_start(wtl[:], W["wqkv_lo"][li, cc].bitcast(FPR))
                wq.append((wth, wtl, cc))
            else:
                wq.append((wth, None, cc))
        for tb in range(NT):
            sl = slice(512 * tb, 512 * (tb + 1))
            xh = []
            xl = []
            for dc in range(8):
                h = asb.tile([128, 512], FPR, tag=f"qxh{dc}", bufs=1,
                             name=u.nm("qxh"))
                src = (xhi_d if split else x_dram)
                nc.sync.dma_start(h[:], src[128 * dc:128 * (dc + 1),
                                            sl].bitcast(FPR))
                xh.append(h)
                if split:
                    l = asb.tile([128, 512], FPR, tag=f"qxl{dc}", bufs=1,
                                 name=u.nm("qxl"))
                    nc.sync.dma_start(l[:], xlo_d[128 * dc:128 * (dc + 1),
                                                  sl].bitcast(FPR))
                    xl.append(l)
            for role in range(2):
                wth, wtl, cc = wq[role]
                acc = ps.tile([128, 512], FP, tag="qacc", bufs=2, name=u.nm("qa"))
                if split:
                    n = 0
                    for dc in range(8):
                        wh = wth[:, 128 * dc:128 * (dc + 1)]
                        wl = wtl[:, 128 * dc:128 * (dc + 1)]
                        for wop, xop in ((wh, xh[dc]), (wh, xl[dc]),
                                         (wl, xh[dc])):
                            nc.tensor.matmul(acc[:], wop, xop[:],
                                             start=(n == 0), stop=(n == 23))
                            n += 1
                else:
                    for dc in range(8):
                        nc.tensor.matmul(acc[:],
                                         wth[:, 128 * dc:128 * (dc + 1)],
                                         xh[dc][:], start=(dc == 0),
                                         stop=(dc == 7))
                tmp = asb.tile([128, 512], FP, tag="qtmp", bufs=3,
                               name=u.nm("qtmp"))
                nc.scalar.activation(tmp[:], acc[:], AF.Identity,
                                     bias=W["bqkv_col"][li][:, cc:cc + 1])
                if split:
                    emit_split(nc, qk[role][0][:, sl], qk[role][1][:, sl],
                               tmp[:])
                else:
                    nc.vector.tensor_copy(qk[role][0][:, sl], tmp[:])
        # scores + AV
        for qb in range(NT):
            sl = slice(512 * qb, 512 * (qb + 1))
            oacc2 = [ps.tile([128, 512], FP, tag="oacc", bufs=2, name=u.nm("oa"))
                     for _ in range(2)]
            for kc in range(NTC):
                ksl = slice(128 * kc, 128 * (kc + 1))
                sp2 = []
                for hh in range(2):
                    hs = slice(64 * hh, 64 * hh + 64)
                    sp = ps.tile([128, 512], FP, tag="sT", bufs=2,
                                 name=u.nm("sT"))
                    if split:
                        nc.tensor.matmul(sp[:], qk[1][0][hs, ksl],
                                         qk[0][0][hs, sl], start=True,
                                         stop=False)
                        nc.tensor.matmul(sp[:], qk[1][0][hs, ksl],
                                         qk[0][1][hs, sl], start=False,
                                         stop=False)
                        nc.tensor.matmul(sp[:], qk[1][1][hs, ksl],
                                         qk[0][0][hs, sl], start=False,
                                         stop=True)
                    else:
                        nc.tensor.matmul(sp[:], qk[1][0][hs, ksl],
                                         qk[0][0][hs, sl], start=True,
                                         stop=True)
                    sp2.append(sp)
                for hh in range(2):
                    pT = asb.tile([128, 512], VDT, tag="pT", bufs=2,
                                  name=u.nm("pT"))
                    nc.scalar.activation(pT[:], sp2[hh][:], AF.Exp, scale=0.125)
                    nc.tensor.matmul(oacc2[hh][0:65, :],
                                     vA[g][kc][:, 65 * hh:65 * hh + 65],
                                     pT[:], start=(kc == 0),
                                     stop=(kc == NTC - 1))
            for hh in range(2):
                hs = slice(64 * hh, 64 * hh + 64)
                rse = asb.tile([1, 512], FP, tag="rse", bufs=2, name=u.nm("rse"))
                nc.vector.reciprocal(rse[:], oacc2[hh][64:65, :])
                bcp = ps.tile([128, 512], FP, tag="bcp", bufs=1, name=u.nm("bcp"))
                nc.tensor.matmul(bcp[0:64, :], ones_row[0:1, 0:64],
                                 rse[:], start=True, stop=True)
                bcs = asb.tile([64, 512], FP, tag="bcs", bufs=2,
                               name=u.nm("bcs"))
                nc.vector.tensor_copy(bcs[:], bcp[0:64, :])
                on = asb.tile([64, 512], FP, tag="on", bufs=2, name=u.nm("on"))
                nc.vector.tensor_tensor(on[:], oacc2[hh][0:64, :],
                                        bcs[:], op=OP.mult)
                nc.scalar.activation(
                    oTn[g][hs, sl], on[:], AF.Identity,
                    bias=W["bv_col"][li][64 * hh:64 * hh + 64, g:g + 1])
        esG.close()
    es0.close()  # vA dead
    esasb.close()

    # ---- Wo ----
    esW = ExitStack()
    wsb = esW.enter_context(tc.tile_pool(name=u.nm("wosb"), bufs=2))
    if split:
        ohl = []
        for dc in range(4):
            hi = wsb.tile([128, Tl], FPR, tag=f"ohi{dc}", bufs=1, name=u.nm("ohi"))
            lo = wsb.tile([128, Tl], FPR, tag=f"olo{dc}", bufs=1, name=u.nm("olo"))
            emit_split(nc, hi[:], lo[:], oTn[dc][:])
            ohl.append((hi[:], lo[:]))
    else:
        ohl = [(oTn[dc][:],) for dc in range(4)]
    woh = []
    wol = []
    for doc in range(8):
        t = wsb.tile([128, 512], FPR, tag=f"woh{doc % 4}", bufs=1, name=u.nm("woh"))
        nc.sync.dma_start(t[:], W["wo_hi"][li, doc].bitcast(FPR))
        woh.append(t)
        if split:
            t2 = wsb.tile([128, 512], FPR, tag=f"wol{doc % 4}", bufs=1,
                          name=u.nm("wol"))
            nc.sync.dma_start(t2[:], W["wo_lo"][li, doc].bitcast(FPR))
            wol.append(t2)
    with tc.tile_pool(name=u.nm("wops"), bufs=1, space="PSUM") as ps:
        for half in range(2):
            for doc in range(4 * half, 4 * half + 4):
                for tb in range(NT):
                    sl = slice(512 * tb, 512 * (tb + 1))
                    acc = ps.tile([128, 512], FP, tag="woacc", bufs=3,
                                  name=u.nm("woa"))
                    n = 0
                    nterm = 12 if split else 4
                    for dc in range(4):
                        wh = woh[doc][:, 128 * dc:128 * (dc + 1)]
                        if split:
                            wl = wol[doc][:, 128 * dc:128 * (dc + 1)]
                            terms = ((wh, ohl[dc][0]), (wh, ohl[dc][1]),
                                     (wl, ohl[dc][0]))
                        else:
                            terms = ((wh, ohl[dc][0]),)
                        for wop, xop in terms:
                            nc.tensor.matmul(acc[:], wop, xop[:, sl],
                                             start=(n == 0),
                                             stop=(n == nterm - 1))
                            n += 1
                    ob = wsb.tile([128, 512], FP, tag="ob", bufs=3, name=u.nm("ob"))
                    nc.scalar.activation(ob[:], acc[:], AF.Identity,
                                         bias=W["bo_col"][li][:, doc:doc + 1])
                    nc.sync.dma_start(ar1[128 * doc:128 * (doc + 1), sl], ob[:])
            if half == 0:
                nc.gpsimd.collective_compute(
                    "AllReduce", OP.add, replica_groups=RG,
                    ins=[ar1[0:512, :]], outs=[ar1o[0:512, :]])
    esW.close()
    esA.close()
    nc.gpsimd.collective_compute("AllReduce", OP.add, replica_groups=RG,
                                 ins=[ar1[512:1024, :]], outs=[ar1o[512:1024, :]])
    xa_d = dram.tile([D, Tl], FP, name=u.nm("xad"))
    emit_ln(nc, tc, u, li, Tl, x_dram, ar1o[:, :], W["ln1g_col"][li],
            W["ln1b_col"][li], C, dram, xa_d[:, :], mode=mode)

    # ---- FFN ----
    ar2 = dram.tile([D, Tl], FP, name=u.nm("ar2i"))
    ar2o = dram.tile([D, Tl], FP, name=u.nm("ar2o"))
    esI = ExitStack()
    fsb = esI.enter_context(tc.tile_pool(name=u.nm("fsb"), bufs=2))
    hp = esI.enter_context(tc.tile_pool(name=u.nm("fh"), bufs=1))
    ps = esI.enter_context(tc.tile_pool(name=u.nm("fps"), bufs=1, space="PSUM"))
    b1c = fsb.tile([128, 16], FP, tag="b1c", bufs=1, name=u.nm("b1c"))
    nc.sync.dma_start(b1c[:], W["b1_col"][li])
    for sb2 in range(NT):  # 512-token slices
        sl = slice(512 * sb2, 512 * (sb2 + 1))
        xah = []
        xal = []
        for dc in range(8):
            xp = fsb.tile([128, 512], FP, tag=f"fx{dc}", bufs=1,
                          name=u.nm("fx"))
            nc.sync.dma_start(xp[:], xa_d[128 * dc:128 * (dc + 1), sl])
            if split:
                h = fsb.tile([128, 512], FPR, tag=f"fxh{dc}", bufs=1,
                             name=u.nm("fxh"))
                l = fsb.tile([128, 512], FPR, tag=f"fxl{dc}", bufs=1,
                             name=u.nm("fxl"))
                emit_split(nc, h[:], l[:], xp[:])
                xah.append(h[:])
                xal.append(l[:])
            else:
                xah.append(xp[:].bitcast(FPR))
        hT = []
        for fc in range(16):
            wth = fsb.tile([128, 1024], FPR, tag="w1h", bufs=2, name=u.nm("w1h"))
            nc.sync.dma_start(wth[:], W["w1_hi"][li, fc].bitcast(FPR))
            if split:
                wtl = fsb.tile([128, 1024], FPR, tag="w1l", bufs=2,
                               name=u.nm("w1l"))
                nc.sync.dma_start(wtl[:], W["w1_lo"][li, fc].bitcast(FPR))
            acc = ps.tile([128, 512], FP, tag="hacc", bufs=3, name=u.nm("ha"))
            n = 0
            nterm = 24 if split else 8
            for dc in range(8):
                wh = wth[:, 128 * dc:128 * (dc + 1)]
                if split:
                    wl = wtl[:, 128 * dc:128 * (dc + 1)]
                    terms = ((wh, xah[dc]), (wh, xal[dc]), (wl, xah[dc]))
                else:
                    terms = ((wh, xah[dc]),)
                for wop, xop in terms:
                    nc.tensor.matmul(acc[:], wop, xop, start=(n == 0),
                                     stop=(n == nterm - 1))
                    n += 1
            if split:
                htmp = fsb.tile([128, 512], FP, tag="htmp", bufs=2,
                                name=u.nm("htmp"))
                nc.scalar.activation(htmp[:], acc[:], AF.Relu,
                                     bias=b1c[:, fc:fc + 1])
                hhi = hp.tile([128, 512], FPR, tag=f"hh{fc}", bufs=1,
                              name=u.nm("hh"))
                hlo = hp.tile([128, 512], FPR, tag=f"hl{fc}", bufs=1,
                              name=u.nm("hl"))
                emit_split(nc, hhi[:], hlo[:], htmp[:])
                hT.append((hhi[:], hlo[:]))
            else:
                hhi = hp.tile([128, 512], FPR, tag=f"hh{fc}", bufs=1,
                              name=u.nm("hh"))
                nc.scalar.activation(hhi[:], acc[:], AF.Relu,
                                     bias=b1c[:, fc:fc + 1])
                hT.append((hhi[:],))
        for doc in range(8):
            wth = fsb.tile([128, 2048], FPR, tag="w2h", bufs=2, name=u.nm("w2h"))
            nc.sync.dma_start(wth[:], W["w2_hi"][li, doc].bitcast(FPR))
            if split:
                wtl = fsb.tile([128, 2048], FPR, tag="w2l", bufs=2,
                               name=u.nm("w2l"))
                nc.sync.dma_start(wtl[:], W["w2_lo"][li, doc].bitcast(FPR))
            acc = ps.tile([128, 512], FP, tag="yacc", bufs=3, name=u.nm("ya"))
            n = 0
            nterm = 48 if split else 16
            for fc in range(16):
                wh = wth[:, 128 * fc:128 * (fc + 1)]
                if split:
                    wl = wtl[:, 128 * fc:128 * (fc + 1)]
                    terms = ((wh, hT[fc][0]), (wh, hT[fc][1]), (wl, hT[fc][0]))
                else:
                    terms = ((wh, hT[fc][0]),)
                for wop, xop in terms:
                    nc.tensor.matmul(acc[:], wop, xop, start=(n == 0),
                                     stop=(n == nterm - 1))
                    n += 1
            yb = fsb.tile([128, 512], FP, tag="yb", bufs=3, name=u.nm("yb"))
            nc.scalar.activation(yb[:], acc[:], AF.Identity,
                                 bias=W["b2_col"][li][:, doc:doc + 1])
            nc.sync.dma_start(ar2[128 * doc:128 * (doc + 1), sl], yb[:])
            if sb2 == NT - 1 and doc == 3:
                nc.gpsimd.collective_compute(
                    "AllReduce", OP.add, replica_groups=RG,
                    ins=[ar2[0:512, :]], outs=[ar2o[0:512, :]])
    esI.close()
    nc.gpsimd.collective_compute("AllReduce", OP.add, replica_groups=RG,
                                 ins=[ar2[512:1024, :]], outs=[ar2o[512:1024, :]])
    emit_ln(nc, tc, u, li, Tl, xa_d[:, :], ar2o[:, :], W["ln2g_col"][li],
            W["ln2b_col"][li], C, dram, out_dram, mode=mode)


def emit_mod(nc, tc, u, li, x_dram, W, C, dram, out_dram, mode="f"):
    xaug = dram.tile([T, 1088], FP, name=u.nm("xaug"))
    srow_d = dram.tile([1, T], FP, name=u.nm("srowd"))
    prow_d = dram.tile([1, T], FP, name=u.nm("prowd"))
    g_d = dram.tile([1, KSEL], I32, name=u.nm("gd"))
    w_d = dram.tile([1, KSEL], FP, name=u.nm("wdd"))
    xsel_d = dram.tile([D, KSEL], FP, name=u.nm("xseld"))
    proc_d = dram.tile([D, KSEL], FP, name=u.nm("procd"))
    gview = g_d[0:1, :].rearrange("a (b p) -> (a b) p", p=128).rearrange("b p -> p b")
    # ---- routing + staging ----
    esA = ExitStack()
    xp = esA.enter_context(tc.tile_pool(name=u.nm("mxin"), bufs=1))
    x_tiles = []
    for dc in range(8):
        t = xp.tile([128, T], FP, tag=f"xin{dc}", bufs=1, name=u.nm("xin"))
        nc.sync.dma_start(t[:], x_dram[128 * dc:128 * (dc + 1), :])
        x_tiles.append(t)
    sb = esA.enter_context(tc.tile_pool(name=u.nm("msb"), bufs=2))
    rowp = esA.enter_context(tc.tile_pool(name=u.nm("mrow"), bufs=1))
    srow = rowp.tile([1, T], FP, tag="srow", bufs=1, name=u.nm("srow"))
    sP = sb.tile([128, 16], FP, tag="sP", bufs=1, name=u.nm("sP"))
    sbc = rowp.tile([128, T], FP, tag="sbc", bufs=1, name=u.nm("sbc"))
    with tc.tile_pool(name=u.nm("mp1"), bufs=1, space="PSUM") as ps:
        for tb in range(4):
            sl = slice(512 * tb, 512 * (tb + 1))
            acc = ps.tile([1, 512], FP, tag="sacc", bufs=2, name=u.nm("sa"))
            for dc in range(8):
                nc.tensor.matmul(acc[:], W["rw_col"][li][:, dc:dc + 1],
                                 x_tiles[dc][:, sl], start=(dc == 0), stop=(dc == 7))
            nc.vector.tensor_copy(srow[0:1, sl], acc[:])
        nc.sync.dma_start(srow_d[0:1, :], srow[:])
        s16 = sb.tile([16, 128], FP, tag="s16", bufs=1, name=u.nm("s16"))
        nc.sync.dma_start(s16[:],
                          srow_d[0:1, :].rearrange("a (b c) -> (a b) c", c=128))
        spp = ps.tile([128, 16], FP, tag="spp", bufs=1, name=u.nm("spp"))
        nc.tensor.transpose(spp[:], s16[:], C["ident"][0:16, 0:16])
        nc.vector.tensor_copy(sP[:], spp[:])
        for tb in range(4):
            sl = slice(512 * tb, 512 * (tb + 1))
            bp = ps.tile([128, 512], FP, tag="bp", bufs=2, name=u.nm("bp"))
            nc.tensor.matmul(bp[:], C["ones_row"][0:1, 0:128], srow[0:1, sl],
                             start=True, stop=True)
            nc.vector.tensor_copy(sbc[:, sl], bp[:])
    rank = rowp.tile([1, T], FP, tag="rank", bufs=1, name=u.nm("rank"))
    with tc.tile_pool(name=u.nm("mp2"), bufs=1, space="PSUM") as ps:
        racc = [ps.tile([1, 512], FP, tag=f"rk{i}", bufs=1, name=u.nm("rk"))
                for i in range(4)]
        for tci in range(16):
            # 0/1 indicators are exact in fp32r -> bit-exact counts at 1 cyc/row
            A = rowp.tile([128, T], FPR, tag="Acmp", bufs=2, name=u.nm("A"))
            with nc.allow_low_precision(reason="exact 0/1 rank counts"):
                nc.vector.tensor_scalar(A[:], sbc[:], sP[:, tci:tci + 1], None,
                                        OP.is_lt)
            for tb in range(4):
                nc.tensor.matmul(racc[tb][:], C["ones_col_r"][:, 0:1],
                                 A[:, 512 * tb:512 * (tb + 1)],
                                 start=(tci == 0), stop=(tci == 15))
        for tb in range(4):
            nc.vector.tensor_copy(rank[0:1, 512 * tb:512 * (tb + 1)], racc[tb][:])
    mask = rowp.tile([1, T], FP, tag="mask", bufs=1, name=u.nm("mask"))
    nc.vector.tensor_scalar(mask[:], rank[:], float(KSEL) - 0.5, None, OP.is_lt)
    zr = rowp.tile([1, T], FP, tag="zr", bufs=1, name=u.nm("zr"))
    nc.vector.memset(zr[:], 0.0)
    pos = rowp.tile([1, T], FP, tag="pos", bufs=1, name=u.nm("pos"))
    nc.vector.tensor_tensor_scan(pos[:], mask[:], zr[:], 0.0, OP.add, OP.add)
    nc.vector.tensor_tensor(pos[:], pos[:], mask[:], op=OP.mult)
    nc.sync.dma_start(prow_d[0:1, :], pos[:])
    with tc.tile_pool(name=u.nm("mp3"), bufs=1, space="PSUM") as ps:
        p16 = sb.tile([16, 128], FP, tag="p16", bufs=1, name=u.nm("p16"))
        nc.sync.dma_start(p16[:],
                          prow_d[0:1, :].rearrange("a (b c) -> (a b) c", c=128))
        ppp = ps.tile([128, 16], FP, tag="ppp", bufs=1, name=u.nm("ppp"))
        nc.tensor.transpose(ppp[:], p16[:], C["ident"][0:16, 0:16])
        posP = sb.tile([128, 16], FP, tag="posP", bufs=1, name=u.nm("posP"))
        nc.vector.tensor_copy(posP[:], ppp[:])
        gacc = [ps.tile([1, 512], FP, tag=f"ga{i}", bufs=1, name=u.nm("ga"))
                for i in range(2)]
        for tci in range(16):
            # 0/1 match indicators and integer token ids (<=2047) are exact
            # in fp32r -> bit-exact gather indices
            R2 = rowp.tile([128, KSEL], FPR, tag="R2", bufs=2, name=u.nm("R2"))
            with nc.allow_low_precision(reason="exact 0/1 gather select"):
                nc.vector.tensor_scalar(R2[:], C["j1bc"][:, 0:KSEL],
                                        posP[:, tci:tci + 1], None, OP.is_equal)
            for gb in range(2):
                nc.tensor.matmul(gacc[gb][:], C["tokid_r"][:, tci:tci + 1],
                                 R2[:, 512 * gb:512 * (gb + 1)],
                                 start=(tci == 0), stop=(tci == 15))
        grow = sb.tile([1, KSEL], FP, tag="grow", bufs=1, name=u.nm("grow"))
        for gb in range(2):
            nc.vector.tensor_copy(grow[0:1, 512 * gb:512 * (gb + 1)], gacc[gb][:])
        gi = sb.tile([1, KSEL], I32, tag="gi", bufs=1, name=u.nm("gi"))
        nc.vector.tensor_copy(gi[:], grow[:])
        nc.sync.dma_start(g_d[0:1, :], gi[:])
    with tc.tile_pool(name=u.nm("mp4"), bufs=1, space="PSUM") as ps:
        for tci in range(16):
            xn = sb.tile([128, 1088], FP, tag="xn", bufs=3, name=u.nm("xn"))
            for dc in range(8):
                tp = ps.tile([128, 128], FP, tag="tp", bufs=4, name=u.nm("tp"))
                nc.tensor.transpose(tp[:], x_tiles[dc][:, 128 * tci:128 * (tci + 1)],
                                    C["ident"][:])
                if dc % 2 == 0:
                    nc.vector.tensor_copy(xn[:, 128 * dc:128 * (dc + 1)], tp[:])
                else:
                    nc.scalar.copy(xn[:, 128 * dc:128 * (dc + 1)], tp[:])
            nc.vector.tensor_copy(xn[:, 1024:1025], sP[:, tci:tci + 1])
            nc.sync.dma_start(xaug[128 * tci:128 * (tci + 1), :], xn[:])
    esA.close()
    # ---- gather selected ----
    with tc.tile_pool(name=u.nm("gsb"), bufs=3) as sb2, \
         tc.tile_pool(name=u.nm("gxs"), bufs=1) as xsp, \
         tc.tile_pool(name=u.nm("gps2"), bufs=1, space="PSUM") as ps:
        xsel = [xsp.tile([128, KSEL], FP, tag=f"sel{i}", bufs=1, name=u.nm("xsel"))
                for i in range(8)]
        wP = sb2.tile([128, 8], FP, tag="wP", bufs=1, name=u.nm("wP"))
        gP = sb2.tile([128, 8], I32, tag="gP2", bufs=1, name=u.nm("gP2"))
        nc.sync.dma_start(gP[:], gview)
        for jc in range(8):
            xg = sb2.tile([128, 1088], FP, tag="xg", bufs=3, name=u.nm("xg"))
            nc.gpsimd.indirect_dma_start(
                xg[:], None, xaug[:, :],
                bass.IndirectOffsetOnAxis(ap=gP[:, jc:jc + 1], axis=0),
                bounds_check=T - 1, oob_is_err=False)
            for dc in range(8):
                tp = ps.tile([128, 128], FP, tag="tp2", bufs=4, name=u.nm("tp2"))
                nc.tensor.transpose(tp[:], xg[:, 128 * dc:128 * (dc + 1)],
                                    C["ident"][:])
                if dc % 2 == 0:
                    nc.vector.tensor_copy(xsel[dc][:, 128 * jc:128 * (jc + 1)], tp[:])
                else:
                    nc.scalar.copy(xsel[dc][:, 128 * jc:128 * (jc + 1)], tp[:])
            nc.scalar.activation(wP[:, jc:jc + 1], xg[:, 1024:1025], AF.Sigmoid)
        wtp = ps.tile([8, 128], FP, tag="wtp", bufs=1, name=u.nm("wtp"))
        nc.tensor.transpose(wtp[:], wP[:], C["ident"][:])
        wts = sb2.tile([8, 128], FP, tag="wts", bufs=1, name=u.nm("wts"))
        nc.vector.tensor_copy(wts[:], wtp[:])
        nc.sync.dma_start(w_d[0:1, :].rearrange("a (b c) -> (a b) c", c=128), wts[:])
        for dc in range(8):
            nc.sync.dma_start(xsel_d[128 * dc:128 * (dc + 1), :], xsel[dc][:])
    # ---- encoder on selected ----
    emit_encoder(nc, tc, u, li, KSEL, xsel_d[:, :], W, C, dram, proc_d[:, :],
                 mode=mode)
    # ---- delta, scatter, rebuild ----
    with tc.tile_pool(name=u.nm("dsb"), bufs=3) as sb3, \
         tc.tile_pool(name=u.nm("dxp"), bufs=1) as dxp, \
         tc.tile_pool(name=u.nm("dps"), bufs=1, space="PSUM") as ps:
        wrow = sb3.tile([1, KSEL], FP, tag="wrow", bufs=1, name=u.nm("wrow"))
        nc.sync.dma_start(wrow[:], w_d[0:1, :])
        gP = sb3.tile([128, 8], I32, tag="gP3", bufs=1, name=u.nm("gP3"))
        nc.sync.dma_start(gP[:], gview)
        wbc = []
        for gb in range(2):
            bp = ps.tile([128, 512], FP, tag="wbp", bufs=2, name=u.nm("wbp"))
            nc.tensor.matmul(bp[:], C["ones_row"][0:1, 0:128],
                             wrow[0:1, 512 * gb:512 * (gb + 1)], start=True, stop=True)
            wb = sb3.tile([128, 512], FP, tag="wbc", bufs=2, name=u.nm("wbc"))
            nc.vector.tensor_copy(wb[:], bp[:])
            wbc.append(wb)
        for dc in range(8):
            xs = dxp.tile([128, KSEL], FP, tag="xs2", bufs=2, name=u.nm("xs2"))
            nc.sync.dma_start(xs[:], xsel_d[128 * dc:128 * (dc + 1), :])
            pr = dxp.tile([128, KSEL], FP, tag="pr2", bufs=2, name=u.nm("pr2"))
            nc.sync.dma_start(pr[:], proc_d[128 * dc:128 * (dc + 1), :])
            ns = dxp.tile([128, KSEL], FP, tag="ns2", bufs=2, name=u.nm("ns2"))
            for gb in range(2):
                sl = slice(512 * gb, 512 * (gb + 1))
                d1 = sb3.tile([128, 512], FP, tag="d1", bufs=2, name=u.nm("d1"))
                nc.vector.tensor_tensor(d1[:], pr[:, sl], xs[:, sl], op=OP.subtract)
                nc.vector.tensor_tensor(d1[:], d1[:], wbc[gb][:], op=OP.mult)
                nc.vector.tensor_tensor(ns[:, sl], d1[:], xs[:, sl], op=OP.add)
            nc.sync.dma_start(proc_d[128 * dc:128 * (dc + 1), :], ns[:])
        for jc in range(8):
            nsl = []
            for dc in range(8):
                t = sb3.tile([128, 128], FP, tag=f"nsl{dc % 4}", bufs=3,
                             name=u.nm("nsl"))
                nc.sync.dma_start(t[:],
                                  proc_d[128 * dc:128 * (dc + 1),
                                         128 * jc:128 * (jc + 1)])
                nsl.append(t)
            nn_ = sb3.tile([128, 1088], FP, tag="nn", bufs=2, name=u.nm("nn"))
            nc.vector.memset(nn_[:, 1024:1088], 0.0)
            for dc in range(8):
                tp = ps.tile([128, 128], FP, tag="tp3", bufs=3, name=u.nm("tp3"))
                nc.tensor.transpose(tp[:], nsl[dc][:], C["ident"][:])
                if dc % 2 == 0:
                    nc.vector.tensor_copy(nn_[:, 128 * dc:128 * (dc + 1)], tp[:])
                else:
                    nc.scalar.copy(nn_[:, 128 * dc:128 * (dc + 1)], tp[:])
            nc.gpsimd.indirect_dma_start(
                xaug[:, :],
                bass.IndirectOffsetOnAxis(ap=gP[:, jc:jc + 1], axis=0),
                nn_[:], None, bounds_check=T - 1, oob_is_err=False)
        for tci in range(16):
            xr = sb3.tile([128, 1024], FP, tag="xrl", bufs=3, name=u.nm("xrl"))
            nc.sync.dma_start(xr[:], xaug[128 * tci:128 * (tci + 1), 0:1024])
            xo = sb3.tile([128, 1024], FP, tag="xo", bufs=3, name=u.nm("xo"))
            for dc in range(8):
                tp = ps.tile([128, 128], FP, tag="tp4", bufs=3, name=u.nm("tp4"))
                nc.tensor.transpose(tp[:], xr[:, 128 * dc:128 * (dc + 1)],
                                    C["ident"][:])
                if dc % 2 == 0:
                    nc.vector.tensor_copy(xo[:, 128 * dc:128 * (dc + 1)], tp[:])
                else:
                    nc.scalar.copy(xo[:, 128 * dc:128 * (dc + 1)], tp[:])
            for dc in range(8):
                nc.sync.dma_start(
                    out_dram[128 * dc:128 * (dc + 1), 128 * tci:128 * (tci + 1)],
                    xo[:, 128 * dc:128 * (dc + 1)])
    return


def build_nc():
    u = Ctr()
    nc = bacc.Bacc("TRN2", target_bir_lowering=False, debug=False, num_devices=8)
    Wd = {}
    for nm, shape in (("wqkv_hi", [NL, 8, 128, 1024]),
                      ("wqkv_lo", [NL, 8, 128, 1024]),
                      ("wo_hi", [NL, 8, 128, 512]),
                      ("wo_lo", [NL, 8, 128, 512]),
                      ("w1_hi", [NL, 16, 128, 1024]),
                      ("w1_lo", [NL, 16, 128, 1024]),
                      ("w2_hi", [NL, 8, 128, 2048]),
                      ("w2_lo", [NL, 8, 128, 2048]),
                      ("wv_rows", [NL, 8, 128, 512]),
                      ("bqkv_col", [NL, 128, 8]),
                      ("bv_col", [NL, 128, 4]),
                      ("bo_col", [NL, 128, 8]),
                      ("b1_col", [NL, 128, 16]),
                      ("b2_col", [NL, 128, 8])):
        Wd[nm] = nc.dram_tensor(nm, shape, FP, kind="ExternalInput")
    for nm in ("ln1g_col", "ln1b_col", "ln2g_col", "ln2b_col", "rw_col"):
        Wd[nm] = nc.dram_tensor(nm, [NL, 128, 8], FP, kind="ExternalInput")
    xT_d = nc.dram_tensor("xT", [D, T], FP, kind="ExternalInput")
    ident_d = nc.dram_tensor("ident", [128, 128], FP, kind="ExternalInput")
    j1bc_d = nc.dram_tensor("j1bc", [128, KSEL], FP, kind="ExternalInput")
    tokid_d = nc.dram_tensor("tokid", [128, 16], FP, kind="ExternalInput")
    out_d = nc.dram_tensor("out_xT", [D, T], FP, kind="ExternalOutput")
    COLW = {"bqkv_col": 8, "bv_col": 4, "bo_col": 8, "b2_col": 8,
            "ln1g_col": 8, "ln1b_col": 8, "ln2g_col": 8, "ln2b_col": 8,
            "rw_col": 8}

    class DramIdx:
        def __init__(self, ap):
            self.ap = ap

        def __getitem__(self, key):
            if isinstance(key, tuple):
                return self.ap[key[0], key[1]]
            return self.ap[key]

    with tile.TileContext(nc) as tc, ExitStack() as ctx, \
         nc.allow_low_precision(reason="split-fp32r keeps fp32 accuracy"):
        cpool = ctx.enter_context(tc.tile_pool(name="consts", bufs=1))
        dram = ctx.enter_context(tc.tile_pool(name="dram", bufs=1, space="DRAM"))
        C = {}
        C["ident"] = cpool.tile([128, 128], FP, tag="ident", bufs=1, name="identc")
        nc.sync.dma_start(C["ident"][:], ident_d[:, :])
        C["ones_row"] = cpool.tile([1, 512], FP, tag="onesr", bufs=1, name="onesr")
        nc.vector.memset(C["ones_row"][:], 1.0)
        C["ones_col"] = cpool.tile([128, 1], FP, tag="onesc", bufs=1, name="onesc")
        nc.vector.memset(C["ones_col"][:], 1.0)
        C["ones_row_r"] = cpool.tile([1, 512], FPR, tag="onesrr", bufs=1,
                                     name="onesrr")
        nc.vector.tensor_copy(C["ones_row_r"][:], C["ones_row"][:])
        C["ones_col_r"] = cpool.tile([128, 1], FPR, tag="onescr", bufs=1,
                                     name="onescr")
        nc.vector.tensor_copy(C["ones_col_r"][:], C["ones_col"][:])
        C["j1bc"] = cpool.tile([128, KSEL], FP, tag="j1bc", bufs=1, name="j1bc")
        nc.sync.dma_start(C["j1bc"][:], j1bc_d[:, :])
        C["tokid"] = cpool.tile([128, 16], FP, tag="tokid", bufs=1, name="tokid")
        nc.sync.dma_start(C["tokid"][:], tokid_d[:, :])
        C["tokid_r"] = cpool.tile([128, 16], FPR, tag="tokidr", bufs=1,
                                  name="tokidr")
        nc.vector.tensor_copy(C["tokid_r"][:], C["tokid"][:])

        W = {}
        for nm in ("wqkv_hi", "wqkv_lo", "wo_hi", "wo_lo", "w1_hi", "w1_lo",
                   "w2_hi", "w2_lo", "wv_rows", "b1_col"):
            W[nm] = DramIdx(Wd[nm])
        for nm, wcol in COLW.items():
            tiles = []
            for li in range(NL):
                t = cpool.tile([128, wcol], FP, tag=f"{nm}{li}",
                               bufs=1, name=f"{nm}{li}")
                nc.sync.dma_start(t[:], Wd[nm][li])
                tiles.append(t)
            W[nm] = tiles

        xd = [dram.tile([D, T], FP, name=f"xd{i}") for i in range(NL + 1)]
        with tc.tile_pool(name="x0p", bufs=1) as x0p:
            for dc in range(8):
                t = x0p.tile([128, T], FP, tag=f"x0{dc}", bufs=1, name=f"x0_{dc}")
                nc.sync.dma_start(t[:], xT_d[128 * dc:128 * (dc + 1), :])
                nc.sync.dma_start(xd[0][128 * dc:128 * (dc + 1), :], t[:])
        nlayers = int(os.environ.get("KLAYERS", NL))
        modes = os.environ.get("KMODES", "fffffr")
        for li in range(nlayers):
            if li % 2 == 1:
                emit_mod(nc, tc, u, li, xd[li][:, :], W, C, dram,
                         xd[li + 1][:, :], mode=modes[li])
            else:
                emit_encoder(nc, tc, u, li, T, xd[li][:, :], W, C, dram,
                             xd[li + 1][:, :], mode=modes[li])
        with tc.tile_pool(name="xfp", bufs=1) as xfp:
            for dc in range(8):
                t = xfp.tile([128, T], FP, tag=f"xf{dc}", bufs=1, name=f"xf_{dc}")
                nc.sync.dma_start(t[:], xd[nlayers][128 * dc:128 * (dc + 1), :])
                nc.sync.dma_start(out_d[128 * dc:128 * (dc + 1), :], t[:])
    nc.compile()
    return nc


def _mask_split(a):
    hi = (a.view(np.uint32) & np.uint32(0xFFFFF000)).view(np.float32)
    return hi, (a - hi).astype(np.float32)


def _pack_inputs(x, Wqkv, bqkv, Wo, bo, W1, b1, W2, b2,
                 ln1g, ln1b, ln2g, ln2b, router_w):
    f32 = np.float32
    maps = []
    ident = np.eye(128, dtype=f32)
    j1bc = np.broadcast_to(np.arange(1, KSEL + 1, dtype=f32), (128, KSEL)).copy()
    tokid = (np.arange(16)[None, :] * 128 + np.arange(128)[:, None]).astype(f32)
    lncols = {
        "ln1g_col": ln1g.reshape(NL, 8, 128).transpose(0, 2, 1).astype(f32).copy(),
        "ln1b_col": ln1b.reshape(NL, 8, 128).transpose(0, 2, 1).astype(f32).copy(),
        "ln2g_col": ln2g.reshape(NL, 8, 128).transpose(0, 2, 1).astype(f32).copy(),
        "ln2b_col": ln2b.reshape(NL, 8, 128).transpose(0, 2, 1).astype(f32).copy(),
        "rw_col": router_w.reshape(NL, 8, 128).transpose(0, 2, 1).astype(f32).copy(),
    }
    for c in range(8):
        p, h = c // 2, c % 2
        fs = slice(DFH * h, DFH * (h + 1))
        m = {"xT": np.ascontiguousarray(x[p].T)}
        wq = np.empty((NL, 8, 128, 1024), f32)
        wvr = np.empty((NL, 8, 128, 512), f32)
        wop = np.empty((NL, 8, 128, 512), f32)
        w1p = np.empty((NL, 16, 128, 1024), f32)
        w2p = np.empty((NL, 8, 128, 2048), f32)
        bqc = np.empty((NL, 128, 8), f32)
        bvc = np.empty((NL, 128, 4), f32)
        boc = np.empty((NL, 128, 8), f32)
        b1c = np.empty((NL, 128, 16), f32)
        b2c = np.empty((NL, 128, 8), f32)
        for l in range(NL):
            Wq = Wqkv[l][512 * h:512 * (h + 1)].T
            Wk = Wqkv[l][D + 512 * h:D + 512 * (h + 1)].T
            Wv = Wqkv[l][2 * D + 512 * h:2 * D + 512 * (h + 1)].T
            qkcat = np.concatenate([Wq, Wk], axis=1)
            for cc in range(8):
                blk = qkcat[:, 128 * cc:128 * (cc + 1)]
                wq[l, cc] = blk.reshape(8, 128, 128).transpose(1, 0, 2).reshape(128, 1024)
            for dc in range(8):
                wvr[l, dc] = Wv[128 * dc:128 * (dc + 1), :]
            WoT_s = Wo[l].T[512 * h:512 * (h + 1), :]
            for doc in range(8):
                blk = WoT_s[:, 128 * doc:128 * (doc + 1)]
                wop[l, doc] = blk.reshape(4, 128, 128).transpose(1, 0, 2).reshape(128, 512)
            W1T_s = W1[l][fs].T
            for fc in range(16):
                blk = W1T_s[:, 128 * fc:128 * (fc + 1)]
                w1p[l, fc] = blk.reshape(8, 128, 128).transpose(1, 0, 2).reshape(128, 1024)
            W2T_s = W2[l].T[fs, :]
            for doc in range(8):
                blk = W2T_s[:, 128 * doc:128 * (doc + 1)]
                w2p[l, doc] = blk.reshape(16, 128, 128).transpose(1, 0, 2).reshape(128, 2048)
            bq = np.concatenate([bqkv[l][:D][512 * h:512 * (h + 1)],
                                 bqkv[l][D:2 * D][512 * h:512 * (h + 1)]])
            bqc[l] = bq.reshape(8, 128).T
            bvc[l] = bqkv[l][2 * D:][512 * h:512 * (h + 1)].reshape(4, 128).T
            boc[l] = (bo[l] * 0.5).reshape(8, 128).T
            b1c[l] = b1[l][fs].reshape(16, 128).T
            b2c[l] = (b2[l] * 0.5).reshape(8, 128).T
        wq_hi, wq_lo = _mask_split(wq)
        wo_hi, wo_lo = _mask_split(wop)
        w1_hi, w1_lo = _mask_split(w1p)
        w2_hi, w2_lo = _mask_split(w2p)
        m.update(wqkv_hi=wq_hi, wqkv_lo=wq_lo, wo_hi=wo_hi, wo_lo=wo_lo,
                 w1_hi=w1_hi, w1_lo=w1_lo, w2_hi=w2_hi, w2_lo=w2_lo,
                 wv_rows=wvr, bqkv_col=bqc, bv_col=bvc, bo_col=boc,
                 b1_col=b1c, b2_col=b2c, ident=ident, j1bc=j1bc, tokid=tokid)
        m.update(lncols)
        maps.append(m)
    return maps


def kernel(**inputs):
    inputs = {k: np.asarray(v, dtype=np.float32) for k, v in inputs.items()}
    if "nc" not in _CACHED:
        _CACHED["nc"] = build_nc()
    nc = _CACHED["nc"]
    maps = _pack_inputs(**inputs)
    kw = {}
    if os.environ.get("KTRACE"):
        kw["trace"] = True
        kw["tmpdir"] = os.environ.get("KTRACE_DIR") or None
    res = bass_utils.run_bass_kernel_spmd(nc, maps, core_ids=list(range(8)), **kw)
    _CACHED["last_res"] = res
    out = np.empty((B, T, D), np.float32)
    for p in range(B):
        out[p] = res.results[2 * p]["out_xT"].T
    return out


# revision 4
# speedup vs baseline: 1.0092x; 1.0092x over previous
"""MixtureOfDepth transformer on 8 trn2 NeuronCores (Bass/Tile).

DP-4 over batch x TP-2 within core pairs. Speed strategy vs the fp32
baseline:
  * All weight GEMMs (QKV, Wo, W1, W2) and attention scores run as 3-term
    split-fp32r matmuls (W = Whi + Wlo, x = xhi + xlo at a 12-significand-bit
    boundary; W@x ~= Whi@xhi + Whi@xlo + Wlo@xhi). fp32r streams at 1
    cycle/row @2.4GHz vs fp32's 2 half-speed passes, so 3 passes beat fp32's
    effective 4 -- at fp32-identical accuracy (validated on HW: 1.7e-7 both).
  * Bias adds via scalar-engine activation bias columns (kills all K=1 bias
    broadcast matmuls).
  * LayerNorm per-token stats math runs in [Tl/128,128] column form (full DVE
    width) instead of [1,512] rows; scale/shift broadcasts are 2-term fp32r.
  * Layer 5 (everything after the last routing decision) runs plain fp32r.
AV (attention @ V) and v-proj stay fp32: their moving operands (softmax
probs) can't be split cheaply. Routing decisions stay exact (fp32 trunk
accuracy everywhere a later top-k depends on it; boundary gaps are as small
as 1.8e-5).
"""
import os, sys
import numpy as np

sys.path.insert(0, "/opt/trn_rl_repo")
import concourse.bass as bass
import concourse.tile as tile
from concourse import bacc, mybir
from concourse import bass_utils
from contextlib import ExitStack

FP = mybir.dt.float32
FPR = mybir.dt.float32r
I32 = mybir.dt.int32

D, H, HD, DFF, NL, T, B = 1024, 16, 64, 4096, 6, 2048, 4
EPS = 1e-5
HH, DFH, KSEL = H // 2, 4096 // 2, T // 2
AF = mybir.ActivationFunctionType
OP = mybir.AluOpType
RG = [[0, 1], [2, 3], [4, 5], [6, 7]]
IMASK = -1048576  # 0xFFFFF000 as int32: keep 12 significand bits

_CACHED = {}


class Ctr:
    def __init__(self):
        self.i = 0

    def nm(self, p):
        self.i += 1
        return f"{p}{self.i}"


def emit_split(nc, hi, lo, src):
    """hi = fp32r-rounding of src (DVE rounding copy); lo = src - hi.

    hi/lo must be fp32r APs, src fp32. src = hi + lo exactly for any
    hardware fp32r mantissa width, and both parts are exactly representable
    in fp32r, so split-term matmuls reconstruct fp32 accuracy."""
    nc.vector.tensor_copy(hi, src)
    nc.vector.tensor_tensor(lo, src.bitcast(FPR), hi, op=OP.subtract)


def emit_ln(nc, tc, u, li, Tl, x_dram, add_dram, g_col, b_col, C, dram,
            out_dram, mode="f"):
    """out <- LN(x + add) * g + b. x/add read from DRAM [D,Tl]; out to DRAM."""
    split = (mode == "f")
    NT = Tl // 512
    NP = Tl // 128
    es = ExitStack()
    sb = es.enter_context(tc.tile_pool(name=u.nm("lnsb"), bufs=2))
    tp = es.enter_context(tc.tile_pool(name=u.nm("lnt"), bufs=1))
    stat_d = dram.tile([1, 2 * Tl], FP, name=u.nm("statd"))
    rs_d = dram.tile([4, Tl], FP, name=u.nm("rsd"))
    t_tiles = []
    esPA = ExitStack()
    psA = esPA.enter_context(tc.tile_pool(name=u.nm("lnpsA"), bufs=1, space="PSUM"))
    a1 = [psA.tile([1, 512], FP, tag=f"r1_{tb}", bufs=1, name=u.nm("r1"))
          for tb in range(NT)]
    a2 = [psA.tile([1, 512], FP, tag=f"r2_{tb}", bufs=1, name=u.nm("r2"))
          for tb in range(NT)]
    ones_col = C["ones_col"]
    for dc in range(8):
        t = tp.tile([128, Tl], FP, tag=f"lt{dc}", bufs=1, name=u.nm("lt"))
        nc.sync.dma_start(t[:], x_dram[128 * dc:128 * (dc + 1), :])
        a = sb.tile([128, Tl], FP, tag="lnadd", bufs=2, name=u.nm("la"))
        nc.sync.dma_start(a[:], add_dram[128 * dc:128 * (dc + 1), :])
        nc.vector.tensor_tensor(t[:], t[:], a[:], op=OP.add)
        x2 = sb.tile([128, Tl], FP, tag="x2", bufs=2, name=u.nm("x2"))
        nc.scalar.square(x2[:], t[:])
        for tb in range(NT):
            sl = slice(512 * tb, 512 * (tb + 1))
            nc.tensor.matmul(a1[tb][:], ones_col[:, 0:1], t[:, sl],
                             start=(dc == 0), stop=(dc == 7))
            nc.tensor.matmul(a2[tb][:], ones_col[:, 0:1], x2[:, sl],
                             start=(dc == 0), stop=(dc == 7))
        t_tiles.append(t)
    # stats to a row, then to column form [2*NP, 128]
    sxq = sb.tile([1, 2 * Tl], FP, tag="sxq", bufs=1, name=u.nm("sxq"))
    for tb in range(NT):
        nc.vector.tensor_copy(sxq[0:1, 512 * tb:512 * (tb + 1)], a1[tb][:])
        nc.vector.tensor_copy(sxq[0:1, Tl + 512 * tb:Tl + 512 * (tb + 1)],
                              a2[tb][:])
    esPA.close()
    nc.sync.dma_start(stat_d[0:1, :], sxq[:])
    sxcol = sb.tile([NP, 128], FP, tag="sxcol", bufs=1, name=u.nm("sxcol"))
    nc.sync.dma_start(sxcol[:],
                      stat_d[0:1, 0:Tl].rearrange("a (b c) -> (a b) c", c=128))
    sqcol = sb.tile([NP, 128], FP, tag="sqcol", bufs=1, name=u.nm("sqcol"))
    nc.sync.dma_start(sqcol[:],
                      stat_d[0:1, Tl:2 * Tl].rearrange("a (b c) -> (a b) c",
                                                       c=128))
    sx = sxcol[:]
    sq = sqcol[:]

    def ctile(nm):
        return sb.tile([NP, 128], FP, tag=nm, bufs=1, name=u.nm(nm))

    mu = ctile("mu")
    nc.vector.tensor_scalar(mu[:], sx, 1.0 / D, None, OP.mult)
    veps = ctile("veps")
    nc.vector.tensor_scalar(veps[:], sq, 1.0 / D, None, OP.mult)
    mu2 = ctile("mu2")
    nc.vector.tensor_tensor(mu2[:], mu[:], mu[:], op=OP.mult)
    nc.vector.tensor_tensor(veps[:], veps[:], mu2[:], op=OP.subtract)
    nc.vector.tensor_scalar(veps[:], veps[:], EPS, None, OP.add)
    s0 = ctile("s0")
    nc.scalar.sqrt(s0[:], veps[:])
    r0 = ctile("r0")
    nc.vector.reciprocal(r0[:], s0[:])
    t1 = ctile("t1")
    nc.vector.tensor_tensor(t1[:], r0[:], r0[:], op=OP.mult)
    nc.vector.tensor_tensor(t1[:], t1[:], veps[:], op=OP.mult)
    nc.vector.tensor_scalar(t1[:], t1[:], -0.5, 1.5, OP.mult, OP.add)
    rs = ctile("rs")
    nc.vector.tensor_tensor(rs[:], r0[:], t1[:], op=OP.mult)
    nmrs = ctile("nmrs")
    nc.vector.tensor_tensor(nmrs[:], mu[:], rs[:], op=OP.mult)
    nc.vector.tensor_scalar(nmrs[:], nmrs[:], -1.0, None, OP.mult)
    # split scale/shift (column form), stage to DRAM, reload as fp32r rows
    def rtile(nm):
        return sb.tile([NP, 128], FPR, tag=nm, bufs=1, name=u.nm(nm))

    if split:
        rsh, rsl, nmh, nml = rtile("rsh"), rtile("rsl"), rtile("nmh"), rtile("nml")
        emit_split(nc, rsh[:], rsl[:], rs[:])
        emit_split(nc, nmh[:], nml[:], nmrs[:])
        quads = [rsh, rsl, nmh, nml]
    else:
        quads = [rtile("rsh"), rtile("nmh")]
        nc.vector.tensor_copy(quads[0][:], rs[:])
        nc.vector.tensor_copy(quads[1][:], nmrs[:])
    rows = []
    for qi, q in enumerate(quads):
        nc.sync.dma_start(
            rs_d[qi:qi + 1, :].rearrange("a (b c) -> (a b) c",
                                         c=128).bitcast(FPR), q[:])
        r = sb.tile([1, Tl], FPR, tag=f"lnrow{qi}", bufs=1, name=u.nm("rows"))
        nc.sync.dma_start(r[:], rs_d[qi:qi + 1, :].bitcast(FPR))
        rows.append(r)
    ones_r = C["ones_row_r"]
    esPB = ExitStack()
    psB = esPB.enter_context(tc.tile_pool(name=u.nm("lnpsB"), bufs=1, space="PSUM"))
    for tb in range(NT):
        sl = slice(512 * tb, 512 * (tb + 1))
        b1p = psB.tile([128, 512], FP, tag="bc1", bufs=2, name=u.nm("b1p"))
        b2p = psB.tile([128, 512], FP, tag="bc2", bufs=2, name=u.nm("b2p"))
        if split:
            nc.tensor.matmul(b1p[:], ones_r[0:1, 0:128], rows[0][0:1, sl],
                             start=True, stop=False)
            nc.tensor.matmul(b1p[:], ones_r[0:1, 0:128], rows[1][0:1, sl],
                             start=False, stop=True)
            nc.tensor.matmul(b2p[:], ones_r[0:1, 0:128], rows[2][0:1, sl],
                             start=True, stop=False)
            nc.tensor.matmul(b2p[:], ones_r[0:1, 0:128], rows[3][0:1, sl],
                             start=False, stop=True)
        else:
            nc.tensor.matmul(b1p[:], ones_r[0:1, 0:128], rows[0][0:1, sl],
                             start=True, stop=True)
            nc.tensor.matmul(b2p[:], ones_r[0:1, 0:128], rows[1][0:1, sl],
                             start=True, stop=True)
        for dc in range(8):
            v1 = sb.tile([128, 512], FP, tag="v1", bufs=3, name=u.nm("v1"))
            nc.vector.tensor_tensor(v1[:], t_tiles[dc][:, sl], b1p[:],
                                    op=OP.mult)
            nc.vector.tensor_tensor(v1[:], v1[:], b2p[:], op=OP.add)
            o1 = sb.tile([128, 512], FP, tag="o1", bufs=3, name=u.nm("o1"))
            nc.scalar.activation(o1[:], v1[:], AF.Identity,
                                 bias=b_col[:, dc:dc + 1],
                                 scale=g_col[:, dc:dc + 1])
            nc.sync.dma_start(out_dram[128 * dc:128 * (dc + 1), sl], o1[:])
    esPB.close()
    es.close()


def emit_encoder(nc, tc, u, li, Tl, x_dram, W, C, dram, out_dram, mode="f"):
    """Encoder layer reading x from DRAM [D, Tl], writing new x to out_dram."""
    split = (mode == "f")
    NT = Tl // 512
    NTC = Tl // 128
    ar1 = dram.tile([D, Tl], FP, name=u.nm("ar1i"))
    ar1o = dram.tile([D, Tl], FP, name=u.nm("ar1o"))
    xhi_d = dram.tile([D, Tl], FP, name=u.nm("xhid"))
    xlo_d = dram.tile([D, Tl], FP, name=u.nm("xlod"))
    ones_row = C["ones_row"]

    # ---- phase 0: v-projection for all groups + x hi/lo staging ----
    esA = ExitStack()
    oTp = esA.enter_context(tc.tile_pool(name=u.nm("aoT"), bufs=1))
    oTn = [oTp.tile([128, Tl], FP if split else FPR, tag=f"oT{i}", bufs=1,
                    name=u.nm("oT"))
           for i in range(4)]
    esasb = ExitStack()
    asb = esasb.enter_context(tc.tile_pool(name=u.nm("asb"), bufs=2))
    es0 = ExitStack()
    vap = es0.enter_context(tc.tile_pool(name=u.nm("avA"), bufs=1))
    VDT = FP if split else FPR
    vA = [[vap.tile([128, 130], VDT, tag=f"vA{g}_{i % 4}",
                    bufs=(NTC + 3) // 4, name=u.nm("vA"))
           for i in range(NTC)] for g in range(4)]
    esV = ExitStack()
    vsb = esV.enter_context(tc.tile_pool(name=u.nm("vsb"), bufs=2))
    vps = esV.enter_context(tc.tile_pool(name=u.nm("vps"), bufs=1, space="PSUM"))
    wv = []
    for dc in range(8):
        t = vsb.tile([128, 512], VDT, tag=f"wv{dc}", bufs=1, name=u.nm("wv"))
        src = W["wv_rows"][li, dc]
        nc.sync.dma_start(t[:], src if split else src.bitcast(FPR))
        wv.append(t)
    ones_c = C["ones_col"]
    for tb in range(NT):
        sl = slice(512 * tb, 512 * (tb + 1))
        xc = []
        for dc in range(8):
            t = vsb.tile([128, 512], FP, tag=f"xc{dc}", bufs=1,
                         name=u.nm("xc"))
            nc.sync.dma_start(t[:], x_dram[128 * dc:128 * (dc + 1), sl])
            xc.append(t)
        if split:
            for dc in range(8):
                hi = vsb.tile([128, 512], FPR, tag="xh", bufs=4,
                              name=u.nm("xh"))
                lo = vsb.tile([128, 512], FPR, tag="xl", bufs=4,
                              name=u.nm("xl"))
                emit_split(nc, hi[:], lo[:], xc[dc][:])
                nc.sync.dma_start(xhi_d[128 * dc:128 * (dc + 1), sl],
                                  hi[:].bitcast(FP))
                nc.sync.dma_start(xlo_d[128 * dc:128 * (dc + 1), sl],
                                  lo[:].bitcast(FP))
        for g in range(4):
            vs = slice(128 * g, 128 * (g + 1))
            for ti in range(4):
                tig = 4 * tb + ti
                acc = vps.tile([128, 128], FP, tag="vacc", bufs=2,
                               name=u.nm("va"))
                for dc in range(8):
                    xop = xc[dc][:, 128 * ti:128 * (ti + 1)]
                    nc.tensor.matmul(acc[:], xop if split else xop.bitcast(FPR),
                                     wv[dc][:, vs], start=(dc == 0),
                                     stop=(dc == 7))
                nc.vector.tensor_copy(vA[g][tig][:, 64:65], ones_c[:, 0:1])
                nc.vector.tensor_copy(vA[g][tig][:, 129:130], ones_c[:, 0:1])
                src = acc[:, :].rearrange("p (h c) -> p h c", c=64)
                dst = vA[g][tig][:, :].rearrange("p (h c) -> p h c",
                                                 c=65)[:, :, 0:64]
                nc.vector.tensor_copy(dst, src)
    esV.close()

    # ---- attention per 2-head group ----
    for g in range(4):
        esG = ExitStack()
        gp = esG.enter_context(tc.tile_pool(name=u.nm("gq"), bufs=1))
        ps = esG.enter_context(tc.tile_pool(name=u.nm("gps"), bufs=1, space="PSUM"))

        if split:
            qk = tuple(
                (gp.tile([128, Tl], FPR, tag=f"q{r}h", bufs=1, name=u.nm("qh")),
                 gp.tile([128, Tl], FPR, tag=f"q{r}l", bufs=1, name=u.nm("ql")))
                for r in range(2))
        else:
            qk = tuple(
                (gp.tile([128, Tl], FPR, tag=f"q{r}h", bufs=1, name=u.nm("qT")),)
                for r in range(2))
        # weights for q (cc=g) and k (cc=4+g)
        wq = []
        for role in range(2):
            cc = g if role == 0 else 4 + g
            wth = asb.tile([128, 1024], FPR, tag=f"wqh{role}", bufs=1,
                           name=u.nm("wqh"))
            nc.sync.dma_start(wth[:], W["wqkv_hi"][li, cc].bitcast(FPR))
            if split:
                wtl = asb.tile([128, 1024], FPR, tag=f"wql{role}", bufs=1,
                               name=u.nm("wql"))
                nc.sync.dma_start(wtl[:], W["wqkv_lo"][li, cc].bitcast(FPR))
                wq.append((wth, wtl, cc))
            else:
                wq.append((wth, None, cc))
        for tb in range(NT):
            sl = slice(512 * tb, 512 * (tb + 1))
            xh = []
            xl = []
            for dc in range(8):
                h = asb.tile([128, 512], FPR, tag=f"qxh{dc}", bufs=1,
                             name=u.nm("qxh"))
                src = (xhi_d if split else x_dram)
                nc.sync.dma_start(h[:], src[128 * dc:128 * (dc + 1),
                                            sl].bitcast(FPR))
                xh.append(h)
                if split:
                    l = asb.tile([128, 512], FPR, tag=f"qxl{dc}", bufs=1,
                                 name=u.nm("qxl"))
                    nc.sync.dma_start(l[:], xlo_d[128 * dc:128 * (dc + 1),
                                                  sl].bitcast(FPR))
                    xl.append(l)
            for role in range(2):
                wth, wtl, cc = wq[role]
                acc = ps.tile([128, 512], FP, tag="qacc", bufs=2, name=u.nm("qa"))
                if split:
                    n = 0
                    for dc in range(8):
                        wh = wth[:, 128 * dc:128 * (dc + 1)]
                        wl = wtl[:, 128 * dc:128 * (dc + 1)]
                        for wop, xop in ((wh, xh[dc]), (wh, xl[dc]),
                                         (wl, xh[dc])):
                            nc.tensor.matmul(acc[:], wop, xop[:],
                                             start=(n == 0), stop=(n == 23))
                            n += 1
                else:
                    for dc in range(8):
                        nc.tensor.matmul(acc[:],
                                         wth[:, 128 * dc:128 * (dc + 1)],
                                         xh[dc][:], start=(dc == 0),
                                         stop=(dc == 7))
                tmp = asb.tile([128, 512], FP, tag="qtmp", bufs=3,
                               name=u.nm("qtmp"))
                nc.scalar.activation(tmp[:], acc[:], AF.Identity,
                                     bias=W["bqkv_col"][li][:, cc:cc + 1])
                if split:
                    emit_split(nc, qk[role][0][:, sl], qk[role][1][:, sl],
                               tmp[:])
                else:
                    nc.vector.tensor_copy(qk[role][0][:, sl], tmp[:])
        # scores + AV, software-pipelined by one kc stage: the PE computes
        # scores for chunk kc+1 while the scalar engine runs exp(kc) and the
        # AV matmuls consume pT(kc) -- hides the scores->exp->AV bubble.
        for qb in range(NT):
            sl = slice(512 * qb, 512 * (qb + 1))
            oacc2 = [ps.tile([128, 512], FP, tag="oacc", bufs=2, name=u.nm("oa"))
                     for _ in range(2)]
            pTs = [None, None]
            for kc in range(NTC + 1):
                if kc < NTC:
                    ksl = slice(128 * kc, 128 * (kc + 1))
                    newpT = []
                    for hh in range(2):
                        hs = slice(64 * hh, 64 * hh + 64)
                        sp = ps.tile([128, 512], FP, tag="sT", bufs=3,
                                     name=u.nm("sT"))
                        if split:
                            nc.tensor.matmul(sp[:], qk[1][0][hs, ksl],
                                             qk[0][0][hs, sl], start=True,
                                             stop=False)
                            nc.tensor.matmul(sp[:], qk[1][0][hs, ksl],
                                             qk[0][1][hs, sl], start=False,
                                             stop=False)
                            nc.tensor.matmul(sp[:], qk[1][1][hs, ksl],
                                             qk[0][0][hs, sl], start=False,
                                             stop=True)
                        else:
                            nc.tensor.matmul(sp[:], qk[1][0][hs, ksl],
                                             qk[0][0][hs, sl], start=True,
                                             stop=True)
                        pT = asb.tile([128, 512], VDT, tag="pT", bufs=4,
                                      name=u.nm("pT"))
                        nc.scalar.activation(pT[:], sp[:], AF.Exp, scale=0.125)
                        newpT.append(pT)
                if kc > 0:
                    for hh in range(2):
                        nc.tensor.matmul(oacc2[hh][0:65, :],
                                         vA[g][kc - 1][:, 65 * hh:65 * hh + 65],
                                         pTs[hh][:], start=(kc == 1),
                                         stop=(kc == NTC))
                if kc < NTC:
                    pTs = newpT
            for hh in range(2):
                hs = slice(64 * hh, 64 * hh + 64)
                rse = asb.tile([1, 512], FP, tag="rse", bufs=2, name=u.nm("rse"))
                nc.vector.reciprocal(rse[:], oacc2[hh][64:65, :])
                bcp = ps.tile([128, 512], FP, tag="bcp", bufs=1, name=u.nm("bcp"))
                nc.tensor.matmul(bcp[0:64, :], ones_row[0:1, 0:64],
                                 rse[:], start=True, stop=True)
                bcs = asb.tile([64, 512], FP, tag="bcs", bufs=2,
                               name=u.nm("bcs"))
                nc.vector.tensor_copy(bcs[:], bcp[0:64, :])
                on = asb.tile([64, 512], FP, tag="on", bufs=2, name=u.nm("on"))
                nc.vector.tensor_tensor(on[:], oacc2[hh][0:64, :],
                                        bcs[:], op=OP.mult)
                nc.scalar.activation(
                    oTn[g][hs, sl], on[:], AF.Identity,
                    bias=W["bv_col"][li][64 * hh:64 * hh + 64, g:g + 1])
        esG.close()
    es0.close()  # vA dead
    esasb.close()

    # ---- Wo ----
    esW = ExitStack()
    wsb = esW.enter_context(tc.tile_pool(name=u.nm("wosb"), bufs=2))
    if split:
        ohl = []
        for dc in range(4):
            hi = wsb.tile([128, Tl], FPR, tag=f"ohi{dc}", bufs=1, name=u.nm("ohi"))
            lo = wsb.tile([128, Tl], FPR, tag=f"olo{dc}", bufs=1, name=u.nm("olo"))
            emit_split(nc, hi[:], lo[:], oTn[dc][:])
            ohl.append((hi[:], lo[:]))
    else:
        ohl = [(oTn[dc][:],) for dc in range(4)]
    woh = []
    wol = []
    for doc in range(8):
        t = wsb.tile([128, 512], FPR, tag=f"woh{doc % 4}", bufs=1, name=u.nm("woh"))
        nc.sync.dma_start(t[:], W["wo_hi"][li, doc].bitcast(FPR))
        woh.append(t)
        if split:
            t2 = wsb.tile([128, 512], FPR, tag=f"wol{doc % 4}", bufs=1,
                          name=u.nm("wol"))
            nc.sync.dma_start(t2[:], W["wo_lo"][li, doc].bitcast(FPR))
            wol.append(t2)
    with tc.tile_pool(name=u.nm("wops"), bufs=1, space="PSUM") as ps:
        for half in range(2):
            for doc in range(4 * half, 4 * half + 4):
                for tb in range(NT):
                    sl = slice(512 * tb, 512 * (tb + 1))
                    acc = ps.tile([128, 512], FP, tag="woacc", bufs=3,
                                  name=u.nm("woa"))
                    n = 0
                    nterm = 12 if split else 4
                    for dc in range(4):
                        wh = woh[doc][:, 128 * dc:128 * (dc + 1)]
                        if split:
                            wl = wol[doc][:, 128 * dc:128 * (dc + 1)]
                            terms = ((wh, ohl[dc][0]), (wh, ohl[dc][1]),
                                     (wl, ohl[dc][0]))
                        else:
                            terms = ((wh, ohl[dc][0]),)
                        for wop, xop in terms:
                            nc.tensor.matmul(acc[:], wop, xop[:, sl],
                                             start=(n == 0),
                                             stop=(n == nterm - 1))
                            n += 1
                    ob = wsb.tile([128, 512], FP, tag="ob", bufs=3, name=u.nm("ob"))
                    nc.scalar.activation(ob[:], acc[:], AF.Identity,
                                         bias=W["bo_col"][li][:, doc:doc + 1])
                    nc.sync.dma_start(ar1[128 * doc:128 * (doc + 1), sl], ob[:])
            if half == 0:
                nc.gpsimd.collective_compute(
                    "AllReduce", OP.add, replica_groups=RG,
                    ins=[ar1[0:512, :]], outs=[ar1o[0:512, :]])
    esW.close()
    esA.close()
    nc.gpsimd.collective_compute("AllReduce", OP.add, replica_groups=RG,
                                 ins=[ar1[512:1024, :]], outs=[ar1o[512:1024, :]])
    xa_d = dram.tile([D, Tl], FP, name=u.nm("xad"))
    emit_ln(nc, tc, u, li, Tl, x_dram, ar1o[:, :], W["ln1g_col"][li],
            W["ln1b_col"][li], C, dram, xa_d[:, :], mode=mode)

    # ---- FFN ----
    ar2 = dram.tile([D, Tl], FP, name=u.nm("ar2i"))
    ar2o = dram.tile([D, Tl], FP, name=u.nm("ar2o"))
    esI = ExitStack()
    fsb = esI.enter_context(tc.tile_pool(name=u.nm("fsb"), bufs=2))
    hp = esI.enter_context(tc.tile_pool(name=u.nm("fh"), bufs=1))
    ps = esI.enter_context(tc.tile_pool(name=u.nm("fps"), bufs=1, space="PSUM"))
    b1c = fsb.tile([128, 16], FP, tag="b1c", bufs=1, name=u.nm("b1c"))
    nc.sync.dma_start(b1c[:], W["b1_col"][li])
    for sb2 in range(NT):  # 512-token slices
        sl = slice(512 * sb2, 512 * (sb2 + 1))
        xah = []
        xal = []
        for dc in range(8):
            xp = fsb.tile([128, 512], FP, tag=f"fx{dc}", bufs=1,
                          name=u.nm("fx"))
            nc.sync.dma_start(xp[:], xa_d[128 * dc:128 * (dc + 1), sl])
            if split:
                h = fsb.tile([128, 512], FPR, tag=f"fxh{dc}", bufs=1,
                             name=u.nm("fxh"))
                l = fsb.tile([128, 512], FPR, tag=f"fxl{dc}", bufs=1,
                             name=u.nm("fxl"))
                emit_split(nc, h[:], l[:], xp[:])
                xah.append(h[:])
                xal.append(l[:])
            else:
                xah.append(xp[:].bitcast(FPR))
        hT = []
        for fc in range(16):
            wth = fsb.tile([128, 1024], FPR, tag="w1h", bufs=2, name=u.nm("w1h"))
            nc.sync.dma_start(wth[:], W["w1_hi"][li, fc].bitcast(FPR))
            if split:
                wtl = fsb.tile([128, 1024], FPR, tag="w1l", bufs=2,
                               name=u.nm("w1l"))
                nc.sync.dma_start(wtl[:], W["w1_lo"][li, fc].bitcast(FPR))
            acc = ps.tile([128, 512], FP, tag="hacc", bufs=3, name=u.nm("ha"))
            n = 0
            nterm = 24 if split else 8
            for dc in range(8):
                wh = wth[:, 128 * dc:128 * (dc + 1)]
                if split:
                    wl = wtl[:, 128 * dc:128 * (dc + 1)]
                    terms = ((wh, xah[dc]), (wh, xal[dc]), (wl, xah[dc]))
                else:
                    terms = ((wh, xah[dc]),)
                for wop, xop in terms:
                    nc.tensor.matmul(acc[:], wop, xop, start=(n == 0),
                                     stop=(n == nterm - 1))
                    n += 1
            if split:
                htmp = fsb.tile([128, 512], FP, tag="htmp", bufs=2,
                                name=u.nm("htmp"))
                nc.scalar.activation(htmp[:], acc[:], AF.Relu,
                                     bias=b1c[:, fc:fc + 1])
                hhi = hp.tile([128, 512], FPR, tag=f"hh{fc}", bufs=1,
                              name=u.nm("hh"))
                hlo = hp.tile([128, 512], FPR, tag=f"hl{fc}", bufs=1,
                              name=u.nm("hl"))
                emit_split(nc, hhi[:], hlo[:], htmp[:])
                hT.append((hhi[:], hlo[:]))
            else:
                hhi = hp.tile([128, 512], FPR, tag=f"hh{fc}", bufs=1,
                              name=u.nm("hh"))
                nc.scalar.activation(hhi[:], acc[:], AF.Relu,
                                     bias=b1c[:, fc:fc + 1])
                hT.append((hhi[:],))
        for doc in range(8):
            wth = fsb.tile([128, 2048], FPR, tag="w2h", bufs=2, name=u.nm("w2h"))
            nc.sync.dma_start(wth[:], W["w2_hi"][li, doc].bitcast(FPR))
            if split:
                wtl = fsb.tile([128, 2048], FPR, tag="w2l", bufs=2,
                               name=u.nm("w2l"))
                nc.sync.dma_start(wtl[:], W["w2_lo"][li, doc].bitcast(FPR))
            acc = ps.tile([128, 512], FP, tag="yacc", bufs=3, name=u.nm("ya"))
            n = 0
            nterm = 48 if split else 16
            for fc in range(16):
                wh = wth[:, 128 * fc:128 * (fc + 1)]
                if split:
                    wl = wtl[:, 128 * fc:128 * (fc + 1)]
                    terms = ((wh, hT[fc][0]), (wh, hT[fc][1]), (wl, hT[fc][0]))
                else:
                    terms = ((wh, hT[fc][0]),)
                for wop, xop in terms:
                    nc.tensor.matmul(acc[:], wop, xop, start=(n == 0),
                                     stop=(n == nterm - 1))
                    n += 1
            yb = fsb.tile([128, 512], FP, tag="yb", bufs=3, name=u.nm("yb"))
            nc.scalar.activation(yb[:], acc[:], AF.Identity,
                                 bias=W["b2_col"][li][:, doc:doc + 1])
            nc.sync.dma_start(ar2[128 * doc:128 * (doc + 1), sl], yb[:])
            if sb2 == NT - 1 and doc == 3:
                nc.gpsimd.collective_compute(
                    "AllReduce", OP.add, replica_groups=RG,
                    ins=[ar2[0:512, :]], outs=[ar2o[0:512, :]])
    esI.close()
    nc.gpsimd.collective_compute("AllReduce", OP.add, replica_groups=RG,
                                 ins=[ar2[512:1024, :]], outs=[ar2o[512:1024, :]])
    emit_ln(nc, tc, u, li, Tl, xa_d[:, :], ar2o[:, :], W["ln2g_col"][li],
            W["ln2b_col"][li], C, dram, out_dram, mode=mode)


def emit_mod(nc, tc, u, li, x_dram, W, C, dram, out_dram, mode="f"):
    xaug = dram.tile([T, 1088], FP, name=u.nm("xaug"))
    srow_d = dram.tile([1, T], FP, name=u.nm("srowd"))
    prow_d = dram.tile([1, T], FP, name=u.nm("prowd"))
    g_d = dram.tile([1, KSEL], I32, name=u.nm("gd"))
    w_d = dram.tile([1, KSEL], FP, name=u.nm("wdd"))
    xsel_d = dram.tile([D, KSEL], FP, name=u.nm("xseld"))
    proc_d = dram.tile([D, KSEL], FP, name=u.nm("procd"))
    gview = g_d[0:1, :].rearrange("a (b p) -> (a b) p", p=128).rearrange("b p -> p b")
    # ---- routing + staging ----
    esA = ExitStack()
    xp = esA.enter_context(tc.tile_pool(name=u.nm("mxin"), bufs=1))
    x_tiles = []
    for dc in range(8):
        t = xp.tile([128, T], FP, tag=f"xin{dc}", bufs=1, name=u.nm("xin"))
        nc.sync.dma_start(t[:], x_dram[128 * dc:128 * (dc + 1), :])
        x_tiles.append(t)
    sb = esA.enter_context(tc.tile_pool(name=u.nm("msb"), bufs=2))
    rowp = esA.enter_context(tc.tile_pool(name=u.nm("mrow"), bufs=1))
    srow = rowp.tile([1, T], FP, tag="srow", bufs=1, name=u.nm("srow"))
    sP = sb.tile([128, 16], FP, tag="sP", bufs=1, name=u.nm("sP"))
    sbc = rowp.tile([128, T], FP, tag="sbc", bufs=1, name=u.nm("sbc"))
    with tc.tile_pool(name=u.nm("mp1"), bufs=1, space="PSUM") as ps:
        for tb in range(4):
            sl = slice(512 * tb, 512 * (tb + 1))
            acc = ps.tile([1, 512], FP, tag="sacc", bufs=2, name=u.nm("sa"))
            for dc in range(8):
                nc.tensor.matmul(acc[:], W["rw_col"][li][:, dc:dc + 1],
                                 x_tiles[dc][:, sl], start=(dc == 0), stop=(dc == 7))
            nc.vector.tensor_copy(srow[0:1, sl], acc[:])
        nc.sync.dma_start(srow_d[0:1, :], srow[:])
        s16 = sb.tile([16, 128], FP, tag="s16", bufs=1, name=u.nm("s16"))
        nc.sync.dma_start(s16[:],
                          srow_d[0:1, :].rearrange("a (b c) -> (a b) c", c=128))
        spp = ps.tile([128, 16], FP, tag="spp", bufs=1, name=u.nm("spp"))
        nc.tensor.transpose(spp[:], s16[:], C["ident"][0:16, 0:16])
        nc.vector.tensor_copy(sP[:], spp[:])
        for tb in range(4):
            sl = slice(512 * tb, 512 * (tb + 1))
            bp = ps.tile([128, 512], FP, tag="bp", bufs=2, name=u.nm("bp"))
            nc.tensor.matmul(bp[:], C["ones_row"][0:1, 0:128], srow[0:1, sl],
                             start=True, stop=True)
            nc.vector.tensor_copy(sbc[:, sl], bp[:])
    rank = rowp.tile([1, T], FP, tag="rank", bufs=1, name=u.nm("rank"))
    with tc.tile_pool(name=u.nm("mp2"), bufs=1, space="PSUM") as ps:
        racc = [ps.tile([1, 512], FP, tag=f"rk{i}", bufs=1, name=u.nm("rk"))
                for i in range(4)]
        for tci in range(16):
            # 0/1 indicators are exact in fp32r -> bit-exact counts at 1 cyc/row
            A = rowp.tile([128, T], FPR, tag="Acmp", bufs=2, name=u.nm("A"))
            with nc.allow_low_precision(reason="exact 0/1 rank counts"):
                nc.vector.tensor_scalar(A[:], sbc[:], sP[:, tci:tci + 1], None,
                                        OP.is_lt)
            for tb in range(4):
                nc.tensor.matmul(racc[tb][:], C["ones_col_r"][:, 0:1],
                                 A[:, 512 * tb:512 * (tb + 1)],
                                 start=(tci == 0), stop=(tci == 15))
        for tb in range(4):
            nc.vector.tensor_copy(rank[0:1, 512 * tb:512 * (tb + 1)], racc[tb][:])
    mask = rowp.tile([1, T], FP, tag="mask", bufs=1, name=u.nm("mask"))
    nc.vector.tensor_scalar(mask[:], rank[:], float(KSEL) - 0.5, None, OP.is_lt)
    zr = rowp.tile([1, T], FP, tag="zr", bufs=1, name=u.nm("zr"))
    nc.vector.memset(zr[:], 0.0)
    pos = rowp.tile([1, T], FP, tag="pos", bufs=1, name=u.nm("pos"))
    nc.vector.tensor_tensor_scan(pos[:], mask[:], zr[:], 0.0, OP.add, OP.add)
    nc.vector.tensor_tensor(pos[:], pos[:], mask[:], op=OP.mult)
    nc.sync.dma_start(prow_d[0:1, :], pos[:])
    with tc.tile_pool(name=u.nm("mp3"), bufs=1, space="PSUM") as ps:
        p16 = sb.tile([16, 128], FP, tag="p16", bufs=1, name=u.nm("p16"))
        nc.sync.dma_start(p16[:],
                          prow_d[0:1, :].rearrange("a (b c) -> (a b) c", c=128))
        ppp = ps.tile([128, 16], FP, tag="ppp", bufs=1, name=u.nm("ppp"))
        nc.tensor.transpose(ppp[:], p16[:], C["ident"][0:16, 0:16])
        posP = sb.tile([128, 16], FP, tag="posP", bufs=1, name=u.nm("posP"))
        nc.vector.tensor_copy(posP[:], ppp[:])
        gacc = [ps.tile([1, 512], FP, tag=f"ga{i}", bufs=1, name=u.nm("ga"))
                for i in range(2)]
        for tci in range(16):
            # 0/1 match indicators and integer token ids (<=2047) are exact
            # in fp32r -> bit-exact gather indices
            R2 = rowp.tile([128, KSEL], FPR, tag="R2", bufs=2, name=u.nm("R2"))
            with nc.allow_low_precision(reason="exact 0/1 gather select"):
                nc.vector.tensor_scalar(R2[:], C["j1bc"][:, 0:KSEL],
                                        posP[:, tci:tci + 1], None, OP.is_equal)
            for gb in range(2):
                nc.tensor.matmul(gacc[gb][:], C["tokid_r"][:, tci:tci + 1],
                                 R2[:, 512 * gb:512 * (gb + 1)],
                                 start=(tci == 0), stop=(tci == 15))
        grow = sb.tile([1, KSEL], FP, tag="grow", bufs=1, name=u.nm("grow"))
        for gb in range(2):
            nc.vector.tensor_copy(grow[0:1, 512 * gb:512 * (gb + 1)], gacc[gb][:])
        gi = sb.tile([1, KSEL], I32, tag="gi", bufs=1, name=u.nm("gi"))
        nc.vector.tensor_copy(gi[:], grow[:])
        nc.sync.dma_start(g_d[0:1, :], gi[:])
    with tc.tile_pool(name=u.nm("mp4"), bufs=1, space="PSUM") as ps:
        for tci in range(16):
            xn = sb.tile([128, 1088], FP, tag="xn", bufs=3, name=u.nm("xn"))
            for dc in range(8):
                tp = ps.tile([128, 128], FP, tag="tp", bufs=4, name=u.nm("tp"))
                nc.tensor.transpose(tp[:], x_tiles[dc][:, 128 * tci:128 * (tci + 1)],
                                    C["ident"][:])
                if dc % 2 == 0:
                    nc.vector.tensor_copy(xn[:, 128 * dc:128 * (dc + 1)], tp[:])
                else:
                    nc.scalar.copy(xn[:, 128 * dc:128 * (dc + 1)], tp[:])
            nc.vector.tensor_copy(xn[:, 1024:1025], sP[:, tci:tci + 1])
            nc.sync.dma_start(xaug[128 * tci:128 * (tci + 1), :], xn[:])
    esA.close()
    # ---- gather selected ----
    with tc.tile_pool(name=u.nm("gsb"), bufs=3) as sb2, \
         tc.tile_pool(name=u.nm("gxs"), bufs=1) as xsp, \
         tc.tile_pool(name=u.nm("gps2"), bufs=1, space="PSUM") as ps:
        xsel = [xsp.tile([128, KSEL], FP, tag=f"sel{i}", bufs=1, name=u.nm("xsel"))
                for i in range(8)]
        wP = sb2.tile([128, 8], FP, tag="wP", bufs=1, name=u.nm("wP"))
        gP = sb2.tile([128, 8], I32, tag="gP2", bufs=1, name=u.nm("gP2"))
        nc.sync.dma_start(gP[:], gview)
        for jc in range(8):
            xg = sb2.tile([128, 1088], FP, tag="xg", bufs=3, name=u.nm("xg"))
            nc.gpsimd.indirect_dma_start(
                xg[:], None, xaug[:, :],
                bass.IndirectOffsetOnAxis(ap=gP[:, jc:jc + 1], axis=0),
                bounds_check=T - 1, oob_is_err=False)
            for dc in range(8):
                tp = ps.tile([128, 128], FP, tag="tp2", bufs=4, name=u.nm("tp2"))
                nc.tensor.transpose(tp[:], xg[:, 128 * dc:128 * (dc + 1)],
                                    C["ident"][:])
                if dc % 2 == 0:
                    nc.vector.tensor_copy(xsel[dc][:, 128 * jc:128 * (jc + 1)], tp[:])
                else:
                    nc.scalar.copy(xsel[dc][:, 128 * jc:128 * (jc + 1)], tp[:])
            nc.scalar.activation(wP[:, jc:jc + 1], xg[:, 1024:1025], AF.Sigmoid)
        wtp = ps.tile([8, 128], FP, tag="wtp", bufs=1, name=u.nm("wtp"))
        nc.tensor.transpose(wtp[:], wP[:], C["ident"][:])
        wts = sb2.tile([8, 128], FP, tag="wts", bufs=1, name=u.nm("wts"))
        nc.vector.tensor_copy(wts[:], wtp[:])
        nc.sync.dma_start(w_d[0:1, :].rearrange("a (b c) -> (a b) c", c=128), wts[:])
        for dc in range(8):
            nc.sync.dma_start(xsel_d[128 * dc:128 * (dc + 1), :], xsel[dc][:])
    # ---- encoder on selected ----
    emit_encoder(nc, tc, u, li, KSEL, xsel_d[:, :], W, C, dram, proc_d[:, :],
                 mode=mode)
    # ---- delta, scatter, rebuild ----
    with tc.tile_pool(name=u.nm("dsb"), bufs=3) as sb3, \
         tc.tile_pool(name=u.nm("dxp"), bufs=1) as dxp, \
         tc.tile_pool(name=u.nm("dps"), bufs=1, space="PSUM") as ps:
        wrow = sb3.tile([1, KSEL], FP, tag="wrow", bufs=1, name=u.nm("wrow"))
        nc.sync.dma_start(wrow[:], w_d[0:1, :])
        gP = sb3.tile([128, 8], I32, tag="gP3", bufs=1, name=u.nm("gP3"))
        nc.sync.dma_start(gP[:], gview)
        wbc = []
        for gb in range(2):
            bp = ps.tile([128, 512], FP, tag="wbp", bufs=2, name=u.nm("wbp"))
            nc.tensor.matmul(bp[:], C["ones_row"][0:1, 0:128],
                             wrow[0:1, 512 * gb:512 * (gb + 1)], start=True, stop=True)
            wb = sb3.tile([128, 512], FP, tag="wbc", bufs=2, name=u.nm("wbc"))
            nc.vector.tensor_copy(wb[:], bp[:])
            wbc.append(wb)
        for dc in range(8):
            xs = dxp.tile([128, KSEL], FP, tag="xs2", bufs=2, name=u.nm("xs2"))
            nc.sync.dma_start(xs[:], xsel_d[128 * dc:128 * (dc + 1), :])
            pr = dxp.tile([128, KSEL], FP, tag="pr2", bufs=2, name=u.nm("pr2"))
            nc.sync.dma_start(pr[:], proc_d[128 * dc:128 * (dc + 1), :])
            ns = dxp.tile([128, KSEL], FP, tag="ns2", bufs=2, name=u.nm("ns2"))
            for gb in range(2):
                sl = slice(512 * gb, 512 * (gb + 1))
                d1 = sb3.tile([128, 512], FP, tag="d1", bufs=2, name=u.nm("d1"))
                nc.vector.tensor_tensor(d1[:], pr[:, sl], xs[:, sl], op=OP.subtract)
                nc.vector.tensor_tensor(d1[:], d1[:], wbc[gb][:], op=OP.mult)
                nc.vector.tensor_tensor(ns[:, sl], d1[:], xs[:, sl], op=OP.add)
            nc.sync.dma_start(proc_d[128 * dc:128 * (dc + 1), :], ns[:])
        for jc in range(8):
            nsl = []
            for dc in range(8):
                t = sb3.tile([128, 128], FP, tag=f"nsl{dc % 4}", bufs=3,
                             name=u.nm("nsl"))
                nc.sync.dma_start(t[:],
                                  proc_d[128 * dc:128 * (dc + 1),
                                         128 * jc:128 * (jc + 1)])
                nsl.append(t)
            nn_ = sb3.tile([128, 1088], FP, tag="nn", bufs=2, name=u.nm("nn"))
            nc.vector.memset(nn_[:, 1024:1088], 0.0)
            for dc in range(8):
                tp = ps.tile([128, 128], FP, tag="tp3", bufs=3, name=u.nm("tp3"))
                nc.tensor.transpose(tp[:], nsl[dc][:], C["ident"][:])
                if dc % 2 == 0:
                    nc.vector.tensor_copy(nn_[:, 128 * dc:128 * (dc + 1)], tp[:])
                else:
                    nc.scalar.copy(nn_[:, 128 * dc:128 * (dc + 1)], tp[:])
            nc.gpsimd.indirect_dma_start(
                xaug[:, :],
                bass.IndirectOffsetOnAxis(ap=gP[:, jc:jc + 1], axis=0),
                nn_[:], None, bounds_check=T - 1, oob_is_err=False)
        for tci in range(16):
            xr = sb3.tile([128, 1024], FP, tag="xrl", bufs=3, name=u.nm("xrl"))
            nc.sync.dma_start(xr[:], xaug[128 * tci:128 * (tci + 1), 0:1024])
            xo = sb3.tile([128, 1024], FP, tag="xo", bufs=3, name=u.nm("xo"))
            for dc in range(8):
                tp = ps.tile([128, 128], FP, tag="tp4", bufs=3, name=u.nm("tp4"))
                nc.tensor.transpose(tp[:], xr[:, 128 * dc:128 * (dc + 1)],
                                    C["ident"][:])
                if dc % 2 == 0:
                    nc.vector.tensor_copy(xo[:, 128 * dc:128 * (dc + 1)], tp[:])
                else:
                    nc.scalar.copy(xo[:, 128 * dc:128 * (dc + 1)], tp[:])
            for dc in range(8):
                nc.sync.dma_start(
                    out_dram[128 * dc:128 * (dc + 1), 128 * tci:128 * (tci + 1)],
                    xo[:, 128 * dc:128 * (dc + 1)])
    return


def build_nc():
    u = Ctr()
    nc = bacc.Bacc("TRN2", target_bir_lowering=False, debug=False, num_devices=8)
    Wd = {}
    for nm, shape in (("wqkv_hi", [NL, 8, 128, 1024]),
                      ("wqkv_lo", [NL, 8, 128, 1024]),
                      ("wo_hi", [NL, 8, 128, 512]),
                      ("wo_lo", [NL, 8, 128, 512]),
                      ("w1_hi", [NL, 16, 128, 1024]),
                      ("w1_lo", [NL, 16, 128, 1024]),
                      ("w2_hi", [NL, 8, 128, 2048]),
                      ("w2_lo", [NL, 8, 128, 2048]),
                      ("wv_rows", [NL, 8, 128, 512]),
                      ("bqkv_col", [NL, 128, 8]),
                      ("bv_col", [NL, 128, 4]),
                      ("bo_col", [NL, 128, 8]),
                      ("b1_col", [NL, 128, 16]),
                      ("b2_col", [NL, 128, 8])):
        Wd[nm] = nc.dram_tensor(nm, shape, FP, kind="ExternalInput")
    for nm in ("ln1g_col", "ln1b_col", "ln2g_col", "ln2b_col", "rw_col"):
        Wd[nm] = nc.dram_tensor(nm, [NL, 128, 8], FP, kind="ExternalInput")
    xT_d = nc.dram_tensor("xT", [D, T], FP, kind="ExternalInput")
    ident_d = nc.dram_tensor("ident", [128, 128], FP, kind="ExternalInput")
    j1bc_d = nc.dram_tensor("j1bc", [128, KSEL], FP, kind="ExternalInput")
    tokid_d = nc.dram_tensor("tokid", [128, 16], FP, kind="ExternalInput")
    out_d = nc.dram_tensor("out_xT", [D, T], FP, kind="ExternalOutput")
    COLW = {"bqkv_col": 8, "bv_col": 4, "bo_col": 8, "b2_col": 8,
            "ln1g_col": 8, "ln1b_col": 8, "ln2g_col": 8, "ln2b_col": 8,
            "rw_col": 8}

    class DramIdx:
        def __init__(self, ap):
            self.ap = ap

        def __getitem__(self, key):
            if isinstance(key, tuple):
                return self.ap[key[0], key[1]]
            return self.ap[key]

    with tile.TileContext(nc) as tc, ExitStack() as ctx, \
         nc.allow_low_precision(reason="split-fp32r keeps fp32 accuracy"):
        cpool = ctx.enter_context(tc.tile_pool(name="consts", bufs=1))
        dram = ctx.enter_context(tc.tile_pool(name="dram", bufs=1, space="DRAM"))
        C = {}
        C["ident"] = cpool.tile([128, 128], FP, tag="ident", bufs=1, name="identc")
        nc.sync.dma_start(C["ident"][:], ident_d[:, :])
        C["ones_row"] = cpool.tile([1, 512], FP, tag="onesr", bufs=1, name="onesr")
        nc.vector.memset(C["ones_row"][:], 1.0)
        C["ones_col"] = cpool.tile([128, 1], FP, tag="onesc", bufs=1, name="onesc")
        nc.vector.memset(C["ones_col"][:], 1.0)
        C["ones_row_r"] = cpool.tile([1, 512], FPR, tag="onesrr", bufs=1,
                                     name="onesrr")
        nc.vector.tensor_copy(C["ones_row_r"][:], C["ones_row"][:])
        C["ones_col_r"] = cpool.tile([128, 1], FPR, tag="onescr", bufs=1,
                                     name="onescr")
        nc.vector.tensor_copy(C["ones_col_r"][:], C["ones_col"][:])
        C["j1bc"] = cpool.tile([128, KSEL], FP, tag="j1bc", bufs=1, name="j1bc")
        nc.sync.dma_start(C["j1bc"][:], j1bc_d[:, :])
        C["tokid"] = cpool.tile([128, 16], FP, tag="tokid", bufs=1, name="tokid")
        nc.sync.dma_start(C["tokid"][:], tokid_d[:, :])
        C["tokid_r"] = cpool.tile([128, 16], FPR, tag="tokidr", bufs=1,
                                  name="tokidr")
        nc.vector.tensor_copy(C["tokid_r"][:], C["tokid"][:])

        W = {}
        for nm in ("wqkv_hi", "wqkv_lo", "wo_hi", "wo_lo", "w1_hi", "w1_lo",
                   "w2_hi", "w2_lo", "wv_rows", "b1_col"):
            W[nm] = DramIdx(Wd[nm])
        for nm, wcol in COLW.items():
            tiles = []
            for li in range(NL):
                t = cpool.tile([128, wcol], FP, tag=f"{nm}{li}",
                               bufs=1, name=f"{nm}{li}")
                nc.sync.dma_start(t[:], Wd[nm][li])
                tiles.append(t)
            W[nm] = tiles

        xd = [dram.tile([D, T], FP, name=f"xd{i}") for i in range(NL + 1)]
        with tc.tile_pool(name="x0p", bufs=1) as x0p:
            for dc in range(8):
                t = x0p.tile([128, T], FP, tag=f"x0{dc}", bufs=1, name=f"x0_{dc}")
                nc.sync.dma_start(t[:], xT_d[128 * dc:128 * (dc + 1), :])
                nc.sync.dma_start(xd[0][128 * dc:128 * (dc + 1), :], t[:])
        nlayers = int(os.environ.get("KLAYERS", NL))
        modes = os.environ.get("KMODES", "fffffr")
        for li in range(nlayers):
            if li % 2 == 1:
                emit_mod(nc, tc, u, li, xd[li][:, :], W, C, dram,
                         xd[li + 1][:, :], mode=modes[li])
            else:
                emit_encoder(nc, tc, u, li, T, xd[li][:, :], W, C, dram,
                             xd[li + 1][:, :], mode=modes[li])
        with tc.tile_pool(name="xfp", bufs=1) as xfp:
            for dc in range(8):
                t = xfp.tile([128, T], FP, tag=f"xf{dc}", bufs=1, name=f"xf_{dc}")
                nc.sync.dma_start(t[:], xd[nlayers][128 * dc:128 * (dc + 1), :])
                nc.sync.dma_start(out_d[128 * dc:128 * (dc + 1), :], t[:])
    nc.compile()
    return nc


def _mask_split(a):
    hi = (a.view(np.uint32) & np.uint32(0xFFFFF000)).view(np.float32)
    return hi, (a - hi).astype(np.float32)


def _pack_inputs(x, Wqkv, bqkv, Wo, bo, W1, b1, W2, b2,
                 ln1g, ln1b, ln2g, ln2b, router_w):
    f32 = np.float32
    maps = []
    ident = np.eye(128, dtype=f32)
    j1bc = np.broadcast_to(np.arange(1, KSEL + 1, dtype=f32), (128, KSEL)).copy()
    tokid = (np.arange(16)[None, :] * 128 + np.arange(128)[:, None]).astype(f32)
    lncols = {
        "ln1g_col": ln1g.reshape(NL, 8, 128).transpose(0, 2, 1).astype(f32).copy(),
        "ln1b_col": ln1b.reshape(NL, 8, 128).transpose(0, 2, 1).astype(f32).copy(),
        "ln2g_col": ln2g.reshape(NL, 8, 128).transpose(0, 2, 1).astype(f32).copy(),
        "ln2b_col": ln2b.reshape(NL, 8, 128).transpose(0, 2, 1).astype(f32).copy(),
        "rw_col": router_w.reshape(NL, 8, 128).transpose(0, 2, 1).astype(f32).copy(),
    }
    for c in range(8):
        p, h = c // 2, c % 2
        fs = slice(DFH * h, DFH * (h + 1))
        m = {"xT": np.ascontiguousarray(x[p].T)}
        wq = np.empty((NL, 8, 128, 1024), f32)
        wvr = np.empty((NL, 8, 128, 512), f32)
        wop = np.empty((NL, 8, 128, 512), f32)
        w1p = np.empty((NL, 16, 128, 1024), f32)
        w2p = np.empty((NL, 8, 128, 2048), f32)
        bqc = np.empty((NL, 128, 8), f32)
        bvc = np.empty((NL, 128, 4), f32)
        boc = np.empty((NL, 128, 8), f32)
        b1c = np.empty((NL, 128, 16), f32)
        b2c = np.empty((NL, 128, 8), f32)
        for l in range(NL):
            Wq = Wqkv[l][512 * h:512 * (h + 1)].T
            Wk = Wqkv[l][D + 512 * h:D + 512 * (h + 1)].T
            Wv = Wqkv[l][2 * D + 512 * h:2 * D + 512 * (h + 1)].T
            qkcat = np.concatenate([Wq, Wk], axis=1)
            for cc in range(8):
                blk = qkcat[:, 128 * cc:128 * (cc + 1)]
                wq[l, cc] = blk.reshape(8, 128, 128).transpose(1, 0, 2).reshape(128, 1024)
            for dc in range(8):
                wvr[l, dc] = Wv[128 * dc:128 * (dc + 1), :]
            WoT_s = Wo[l].T[512 * h:512 * (h + 1), :]
            for doc in range(8):
                blk = WoT_s[:, 128 * doc:128 * (doc + 1)]
                wop[l, doc] = blk.reshape(4, 128, 128).transpose(1, 0, 2).reshape(128, 512)
            W1T_s = W1[l][fs].T
            for fc in range(16):
                blk = W1T_s[:, 128 * fc:128 * (fc + 1)]
                w1p[l, fc] = blk.reshape(8, 128, 128).transpose(1, 0, 2).reshape(128, 1024)
            W2T_s = W2[l].T[fs, :]
            for doc in range(8):
                blk = W2T_s[:, 128 * doc:128 * (doc + 1)]
                w2p[l, doc] = blk.reshape(16, 128, 128).transpose(1, 0, 2).reshape(128, 2048)
            bq = np.concatenate([bqkv[l][:D][512 * h:512 * (h + 1)],
                                 bqkv[l][D:2 * D][512 * h:512 * (h + 1)]])
            bqc[l] = bq.reshape(8, 128).T
            bvc[l] = bqkv[l][2 * D:][512 * h:512 * (h + 1)].reshape(4, 128).T
            boc[l] = (bo[l] * 0.5).reshape(8, 128).T
            b1c[l] = b1[l][fs].reshape(16, 128).T
            b2c[l] = (b2[l] * 0.5).reshape(8, 128).T
        wq_hi, wq_lo = _mask_split(wq)
        wo_hi, wo_lo = _mask_split(wop)
        w1_hi, w1_lo = _mask_split(w1p)
        w2_hi, w2_lo = _mask_split(w2p)
        m.update(wqkv_hi=wq_hi, wqkv_lo=wq_lo, wo_hi=wo_hi, wo_lo=wo_lo,
                 w1_hi=w1_hi, w1_lo=w1_lo, w2_hi=w2_hi, w2_lo=w2_lo,
                 wv_rows=wvr, bqkv_col=bqc, bv_col=bvc, bo_col=boc,
                 b1_col=b1c, b2_col=b2c, ident=ident, j1bc=j1bc, tokid=tokid)
        m.update(lncols)
        maps.append(m)
    return maps


def kernel(**inputs):
    inputs = {k: np.asarray(v, dtype=np.float32) for k, v in inputs.items()}
    if "nc" not in _CACHED:
        _CACHED["nc"] = build_nc()
    nc = _CACHED["nc"]
    maps = _pack_inputs(**inputs)
    kw = {}
    if os.environ.get("KTRACE"):
        kw["trace"] = True
        kw["tmpdir"] = os.environ.get("KTRACE_DIR") or None
    res = bass_utils.run_bass_kernel_spmd(nc, maps, core_ids=list(range(8)), **kw)
    _CACHED["last_res"] = res
    out = np.empty((B, T, D), np.float32)
    for p in range(B):
        out[p] = res.results[2 * p]["out_xT"].T
    return out
